# revision 29
# baseline (speedup 1.0000x reference)
"""Attention-LSTM captioning model on 8 trn2 cores (8-way tensor parallel).

Gate/itr/vocab output dims sharded across cores (full B=64 per core);
attention batch-sharded (8 batches/core, selected via per-core one-hot bsel
data, since the SPMD program is identical on every core). Activations are
transposed [feature, batch]. Per step: AllGather(att_res), AllGather(nh
chunk). Sigmoid(x) is computed as (tanh(x/2)+1)/2 so phase 1 only needs the
{tanh, exp} ACT table; the hidden state is stored as 2*h with h-consuming
weights pre-halved on the host.

v2: att_res matmul runs in fp8e4 DoubleRow mode (w scaled x32, a2c
pre-divided); the logit matmul is computed per step-pair with M=128 (both
steps' h in one stationary tile), scheduled one pair late so it lands in
the att_res AllGather window; logits stay in SBUF as bf16 and log_softmax
stats are folded into phase 1, so phase 2 is just one stats AllGather and
the final normalize (no DRAM scratch roundtrip).
"""
import numpy as np
import ml_dtypes

import concourse.bacc as bacc
import concourse.mybir as mybir
import concourse.tile as tile
from concourse.ap import AP
from concourse.bass_utils import run_bass_kernel_spmd

BF16_NP = ml_dtypes.bfloat16
FP8_NP = ml_dtypes.float8_e4m3
FP32 = mybir.dt.float32
BF16 = mybir.dt.bfloat16
FP8 = mybir.dt.float8e4
AF = mybir.ActivationFunctionType
ALU = mybir.AluOpType
AX = mybir.AxisListType
PM = mybir.MatmulPerfMode

B, T, R, H, F, E, L, V1 = 64, 20, 1024, 512, 2048, 300, 196, 12001
NC = 8
BMY = B // NC
GC = R // NC              # 128
NGATE = 5 * GC            # 640
VP = 1504
NG = 14                   # l-groups of 16 (224 >= L), even for fp8 pairs
LP = NG * 16              # 224
W_SCALE = 32.0
EP = 384
HCN = H // 128            # 4
FCN = F // 128            # 16
RCN = R // 128            # 8
NBL = BMY * L             # 1568


def _bf(x):
    return np.ascontiguousarray(np.asarray(x, dtype=np.float32)).astype(BF16_NP)


def _f8(x):
    return np.ascontiguousarray(np.asarray(x, dtype=np.float32)).astype(FP8_NP)


def bcast_free(ap, n):
    """Append a step-0 free dim of size n to an AP (broadcast)."""
    return AP(ap.tensor, ap.offset, list(ap.ap) + [[0, n]])


def host_prep(inputs):
    seq = np.asarray(inputs["seq"])
    att = np.asarray(inputs["att_feats"], dtype=np.float32)
    embed_w = np.asarray(inputs["embed_w"], dtype=np.float32)
    ctx2att_w = np.asarray(inputs["ctx2att_w"], dtype=np.float32)
    ctx2att_b = np.asarray(inputs["ctx2att_b"], dtype=np.float32)
    h2att_w = np.asarray(inputs["h2att_w"], dtype=np.float32)
    h2att_b = np.asarray(inputs["h2att_b"], dtype=np.float32)
    alpha_w = np.asarray(inputs["alpha_w"], dtype=np.float32)
    i2h_w = np.asarray(inputs["i2h_w"], dtype=np.float32)
    i2h_b = np.asarray(inputs["i2h_b"], dtype=np.float32)
    h2h_w = np.asarray(inputs["h2h_w"], dtype=np.float32)
    h2h_b = np.asarray(inputs["h2h_b"], dtype=np.float32)
    a2c_w = np.asarray(inputs["a2c_w"], dtype=np.float32)
    a2c_b = np.asarray(inputs["a2c_b"], dtype=np.float32)
    logit_w = np.asarray(inputs["logit_w"], dtype=np.float32)
    logit_b = np.asarray(inputs["logit_b"], dtype=np.float32)

    xt = embed_w[seq]                                    # [B, T, E]
    xtT = np.zeros((EP, T * B), dtype=np.float32)
    xtT[:E] = xt.transpose(2, 1, 0).reshape(E, T * B)
    xtT[E] = 1.0
    xtT = _bf(xtT)
    bias_gate = i2h_b + h2h_b

    in_maps = []
    for c in range(NC):
        m = {"xtT": xtT}
        grows = np.concatenate([np.arange(gg * R + c * GC, gg * R + (c + 1) * GC)
                                for gg in range(5)])
        i2hT = np.zeros((EP, NGATE), dtype=np.float32)
        i2hT[:E] = i2h_w[grows, :].T
        i2hT[E] = bias_gate[grows]
        m["i2hT"] = _bf(i2hT)
        m["h2hT"] = _bf(h2h_w[grows, :].T * 0.5)
        ht8 = (h2att_w.T * 0.5).reshape(4, 2, 128, H)
        m["h2attT_f8"] = _f8(ht8.transpose(0, 2, 1, 3).reshape(512, 2 * H))
        m["h2att_bias"] = _bf(h2att_b[None, :])
        m["ctxT"] = _bf(ctx2att_w.T)
        m["ctx_bias"] = _bf(ctx2att_b[None, :])
        amy = att[c * BMY:(c + 1) * BMY]                 # [8, L, F]
        m["attT_f8"] = _f8(amy.transpose(2, 0, 1).reshape(F, NBL))
        # ctx2att in fp8 k-tile-pair layout: [pair*128+p, i*H+h] =
        # ctx2att_w.T[pair*256+i*128+p, h]
        cT = ctx2att_w.T.reshape(8, 2, 128, H)           # [pair, i, p, h]
        m["ctxT_f8"] = _f8(cT.transpose(0, 2, 1, 3).reshape(1024, 2 * H))
        # fp8 att for the att_res matmul: part p=(b*16+lg), col=(g*F+f),
        # value att[b, g*16+lg, f]
        apad = np.zeros((BMY, LP, F), dtype=np.float32)
        apad[:, :L] = amy
        m["att_f8"] = _f8(apad.reshape(BMY, NG, 16, F).transpose(0, 2, 1, 3)
                          .reshape(128, NG * F))
        # alpha one-hot diag, fp8 k-pair layout per hc-pair:
        # [hcp*128+p, b*32+i*16+m] = 32*alpha[(2hcp+i)*128+p] iff m==b
        ac = np.zeros((2, 128, BMY, 2, 16), dtype=np.float32)
        for hcp in range(2):
            for i in range(2):
                for b in range(BMY):
                    ac[hcp, :, b, i, b] = \
                        W_SCALE * alpha_w[0, (2 * hcp + i) * 128:
                                          (2 * hcp + i + 1) * 128]
        m["alpha_f8"] = _f8(ac.reshape(256, BMY * 32))
        arows = np.concatenate([np.arange(c * GC, (c + 1) * GC),
                                np.arange(R + c * GC, R + (c + 1) * GC)])
        # a2c in fp8 k-pair layout, scaled x16 (itr add divides by 512)
        a2cp = (a2c_w[arows, :].T * 16.0).reshape(8, 2, 128, 256)
        m["a2cT_f8"] = _f8(a2cp.transpose(0, 2, 1, 3).reshape(1024, 512))
        m["a2c_bias"] = _bf(a2c_b[arows][None, :] * 512.0)
        vrows = np.arange(c * VP, (c + 1) * VP)
        lw = np.zeros((R, VP), dtype=np.float32)
        lb = np.full((1, VP), -1e30, dtype=np.float32)
        valid = vrows < V1
        lw[:, valid] = logit_w[vrows[valid], :].T * 0.5
        lb[0, valid] = logit_b[vrows[valid]]
        # fp8 k-pair layout, x8 for fp8 range (bias-add divides by 8)
        lwp = (lw * 8.0).reshape(4, 2, 128, VP)
        m["logitT_f8"] = _f8(lwp.transpose(0, 2, 1, 3).reshape(512, 2 * VP))
        m["logit_bias"] = lb
        m["ident"] = _bf(np.eye(128))
        bsel = np.zeros((B, BMY), dtype=np.float32)
        for j in range(BMY):
            bsel[c * BMY + j, j] = 1.0
        m["bsel"] = _bf(bsel)
        in_maps.append(m)
    return in_maps


def build(t_steps=T, probes=(), reps=1, no_cc=False):
    assert t_steps % 2 == 0
    nc = bacc.Bacc("TRN2", target_bir_lowering=False, debug=False,
                   num_devices=NC)
    probes = set(probes)
    NT = t_steps * B // 128
    NPAIR = t_steps // 2
    RG = [list(range(NC))]

    def din(name, shape, dt=BF16):
        return nc.dram_tensor(name, shape, dt, kind="ExternalInput")

    xtT_d = din("xtT", [EP, T * B])
    i2hT_d = din("i2hT", [EP, NGATE])
    h2hT_d = din("h2hT", [R, NGATE])
    h2attT_d = din("h2attT_f8", [512, 2 * H], FP8)
    h2att_b_d = din("h2att_bias", [1, H])
    ctxT_d = din("ctxT_f8", [1024, 2 * H], FP8)
    ctx_b_d = din("ctx_bias", [1, H])
    attT_d = din("attT_f8", [F, NBL], FP8)
    att_f8_d = din("att_f8", [128, NG * F], FP8)
    alpha_d = din("alpha_f8", [256, BMY * 32], FP8)
    a2cT_d = din("a2cT_f8", [1024, 512], FP8)
    a2c_b_d = din("a2c_bias", [1, 256])
    logitT_d = din("logitT_f8", [512, 2 * VP], FP8)
    logit_b_d = din("logit_bias", [1, VP], FP32)
    ident_d = din("ident", [128, 128])
    bsel_d = din("bsel", [B, BMY])

    out_d = nc.dram_tensor("logp", [t_steps * B, VP], FP32,
                           kind="ExternalOutput")
    agA_out_r = [[nc.dram_tensor(f"agA_out_{rp}_{t}", [B, F], BF16,
                                 addr_space="Shared") for t in range(t_steps)]
                 for rp in range(reps)]
    agH_out_r = [[nc.dram_tensor(f"agH_out_{rp}_{t}", [R, B], BF16,
                                 addr_space="Shared") for t in range(t_steps)]
                 for rp in range(reps)]
    agS_out_r = [nc.dram_tensor(f"agS_out_{rp}", [NC * 128, 2 * NT], FP32,
                                addr_space="Shared") for rp in range(reps)]

    with tile.TileContext(nc) as tc:
        with (
            tc.tile_pool(name="wpool", bufs=1) as wpool,
            tc.tile_pool(name="hpool", bufs=3) as hpool,
            tc.tile_pool(name="psum", bufs=1, space="PSUM") as psum,
            tc.tile_pool(name="dram", bufs=4, space="DRAM") as dpool,
        ):
            def probe_(name, src_ap, shape, dt):
                pd = nc.dram_tensor(f"probe_{name}", list(shape), dt,
                                    kind="ExternalOutput")
                nc.sync.dma_start(out=pd[:], in_=src_ap)

            def load_chunks(pool, dram, cols, n, tag, dt=BF16):
                ts = []
                for i in range(n):
                    t_ = pool.tile([128, cols], dt, tag=f"{tag}{i}",
                                   name=f"{tag}{i}")
                    nc.sync.dma_start(out=t_[:],
                                      in_=dram[i * 128:(i + 1) * 128, :])
                    ts.append(t_)
                return ts

            logitT_s = load_chunks(wpool, logitT_d, 2 * VP, 4, "logitT", FP8)
            logit_b_s = wpool.tile([128, VP], FP32, tag="logitb",
                                   name="logitb")
            _lb_src = AP(logit_b_d[:].tensor, logit_b_d[:].offset,
                         [[0, 128], [1, VP]])
            nc.sync.dma_start(out=logit_b_s[:], in_=_lb_src)
            ident_s = wpool.tile([128, 128], BF16, tag="ident", name="ident")
            nc.sync.dma_start(out=ident_s[:], in_=ident_d[:])
            ones64 = wpool.tile([1, B], BF16, tag="ones64", name="ones64")
            nc.vector.memset(ones64[:], 1.0)
            negm_all = wpool.tile([128, NT], FP32, tag="negm_all",
                                  name="negm_all")
            s_all = wpool.tile([128, NT], FP32, tag="s_all", name="s_all")
            # bf16 logits resident in SBUF, one tile per step-pair
            lgb = [wpool.tile([128, VP], BF16, tag=f"lgb{k}", name=f"lgb{k}")
                   for k in range(NPAIR)]

            with tc.tile_pool(name="w1pool", bufs=1) as w1pool:
                xtT_s = load_chunks(w1pool, xtT_d, T * B, 3, "xtT")
                i2hT_s = load_chunks(w1pool, i2hT_d, NGATE, 3, "i2hT")
                h2hT_s = load_chunks(w1pool, h2hT_d, NGATE, RCN, "h2hT")
                h2attT_s = load_chunks(w1pool, h2attT_d, 2 * H, 4,
                                       "h2attT", FP8)
                att_f8_s = w1pool.tile([128, NG * F], FP8, tag="attf8",
                                       name="attf8")
                nc.sync.dma_start(out=att_f8_s[:], in_=att_f8_d[:])
                alpha_s = load_chunks(w1pool, alpha_d, BMY * 32, 2,
                                      "alpha", FP8)
                a2cT_s = load_chunks(w1pool, a2cT_d, 512, 8, "a2cT", FP8)
                bsel_s = w1pool.tile([B, BMY], BF16, tag="bsel", name="bsel")
                nc.sync.dma_start(out=bsel_s[:], in_=bsel_d[:])
                h2att_b_s = w1pool.tile([1, H], BF16, tag="h2attb",
                                        name="h2attb")
                nc.sync.dma_start(out=h2att_b_s[:], in_=h2att_b_d[:])
                ctx_b_s = w1pool.tile([1, H], BF16, tag="ctxb", name="ctxb")
                nc.sync.dma_start(out=ctx_b_s[:], in_=ctx_b_d[:])
                a2c_b_s = w1pool.tile([1, 256], BF16, tag="a2cb", name="a2cb")
                nc.sync.dma_start(out=a2c_b_s[:], in_=a2c_b_d[:])
                onesNBL = w1pool.tile([1, NBL], BF16, tag="onesNBL",
                                      name="onesNBL")
                nc.vector.memset(onesNBL[:], 1.0)
                p_attT = [w1pool.tile([128, NBL], BF16, tag=f"pattT{hc}",
                                      name=f"pattT{hc}")
                          for hc in range(HCN)]
                stat_all = w1pool.tile([128, LP], FP8, tag="stat_all",
                                       name="stat_all")
                nc.vector.memset(stat_all[:], 0.0)
                w_f8 = w1pool.tile([BMY, LP], FP8, tag="w_f8", name="w_f8")
                nc.vector.memset(w_f8[:], 0.0)
                zh = w1pool.tile([128, 64], BF16, tag="zh", name="zh")
                nc.vector.memset(zh[:], 0.0)
                zf8 = w1pool.tile([128, 384], FP8, tag="zf8", name="zf8")
                nc.vector.memset(zf8[:], 0.0)
                c_st = w1pool.tile([B, GC], FP32, tag="c_st", name="c_st")

                def emit_rep(rep):
                    agA_out = agA_out_r[rep]
                    agH_out = agH_out_r[rep]
                    agS_out = agS_out_r[rep]

                    def probe(name, src_ap, shape, dt):
                        if rep == 0 and name in probes:
                            probe_(name, src_ap, shape, dt)

                    nc.vector.memset(c_st[:], 0.0)
                    # h2 blocks: [128, rc(8) x half(2) x b(64)]; block k
                    # holds h_{2k+1} (half 0) and h_{2k+2} (half 1), as 2*h.
                    # h2f8 blocks mirror them in fp8 for ah/logit matmuls.
                    h2b = [None] * NPAIR
                    h2f8b = [None] * NPAIR

                    def h_ap(j, rc):
                        """lhsT slice [128, 64] for h_j, R-chunk rc."""
                        if j == 0:
                            return zh[:]
                        blk = h2b[(j - 1) // 2]
                        c0 = rc * 128 + ((j - 1) % 2) * 64
                        return blk[:, c0:c0 + 64]

                    # ---------- phase 0 ----------
                    with (
                        tc.tile_pool(name=f"ctxpool{rep}", bufs=1) as ctxpool,
                        tc.tile_pool(name=f"stream{rep}", bufs=3) as stream,
                    ):
                        ctxT_s = load_chunks(ctxpool, ctxT_d, 2 * H, 8,
                                             "ctxT", FP8)
                        QW = 392
                        for q in range(4):
                            n0 = q * QW
                            _pa_tags = ["sums", "ah", "ar", "lg"]
                            pa_ps = [psum.tile([128, QW], FP32,
                                               tag=_pa_tags[hc],
                                               name=f"pa{hc}",
                                               bufs=(2 if hc == 3 else 1))
                                     for hc in range(HCN)]
                            for fc2 in range(8):
                                at = stream.tile([128, 2 * QW], FP8,
                                                 tag="attTq", name="attTq")
                                nc.sync.dma_start(
                                    out=at[:].rearrange(
                                        "p (two n) -> p two n", two=2),
                                    in_=attT_d[fc2 * 256:(fc2 + 1) * 256,
                                               n0:n0 + QW].rearrange(
                                        "(two p) n -> p two n", two=2))
                                for hc in range(HCN):
                                    lhs = AP(ctxT_s[fc2][:].tensor,
                                             ctxT_s[fc2][:].offset
                                             + hc * 128,
                                             [list(ctxT_s[fc2][:].ap[0]),
                                              [H, 2], [1, 128]])
                                    nc.tensor.matmul(
                                        pa_ps[hc][:], lhs,
                                        at[:].rearrange(
                                            "p (two n) -> p two n", two=2),
                                        start=(fc2 == 0), stop=False,
                                        perf_mode=PM.DoubleRow)
                            for hc in range(HCN):
                                nc.tensor.matmul(
                                    pa_ps[hc][:],
                                    ctx_b_s[:, hc * 128:(hc + 1) * 128],
                                    onesNBL[:, n0:n0 + QW], start=False,
                                    stop=True)
                                nc.vector.tensor_copy(
                                    p_attT[hc][:, n0:n0 + QW], pa_ps[hc][:])
                    probe("p_attT0", p_attT[0][:], [128, NBL], BF16)

                    def emit_logit_pair(k):
                        """Logit matmul for step pair k from h2b[k] (M=128),
                        bias-add into lgb[k] (bf16)."""
                        fblk = h2f8b[k]
                        for ci, c0 in enumerate((0, 512, 1024)):
                            c1 = min(VP, c0 + 512)
                            lg_ps = psum.tile([128, 512], FP32, tag="lg",
                                              name="lg_ps", bufs=2)
                            for rc2 in range(4):
                                lhs = AP(fblk[:].tensor,
                                         fblk[:].offset + rc2 * 256,
                                         [list(fblk[:].ap[0]),
                                          [128, 2], [1, 128]])
                                rhs = AP(logitT_s[rc2][:].tensor,
                                         logitT_s[rc2][:].offset + c0,
                                         [list(logitT_s[rc2][:].ap[0]),
                                          [VP, 2], [1, c1 - c0]])
                                nc.tensor.matmul(
                                    lg_ps[:, 0:c1 - c0], lhs, rhs,
                                    start=(rc2 == 0), stop=(rc2 == 3),
                                    perf_mode=PM.DoubleRow)
                            nc.vector.scalar_tensor_tensor(
                                lgb[k][:, c0:c1], lg_ps[:, 0:c1 - c0],
                                1.0 / 8.0, logit_b_s[:, c0:c1],
                                op0=ALU.mult, op1=ALU.add)

                    def emit_pair_stats(k, work):
                        """Softmax stats for pair k (reads lgb[k], SBUF)."""
                        nc.vector.tensor_reduce(
                            negm_all[:, k:k + 1], lgb[k][:], axis=AX.X,
                            op=ALU.max, negate=True)
                        junk = work.tile([128, VP], BF16, tag="p2junk",
                                         name="p2junk", bufs=2)
                        nc.scalar.activation(
                            junk[:], lgb[k][:], AF.Exp,
                            bias=negm_all[:, k:k + 1],
                            accum_out=s_all[:, k:k + 1])

                    # ---------- phase 1 ----------
                    with tc.tile_pool(name=f"work1_{rep}", bufs=1) as work:
                        for t in range(t_steps):
                            # stats for the pair computed two steps ago run
                            # in this step's sums/ah window (DVE+ACT idle)
                            if t >= 3 and t % 2 == 1:
                                emit_pair_stats((t - 3) // 2, work)
                            sums_ps = psum.tile([B, NGATE], FP32, tag="sums",
                                                name="sums", bufs=1)
                            for c0 in (0, 512):
                                c1 = min(NGATE, c0 + 512)
                                for kc in range(3):
                                    nc.tensor.matmul(
                                        sums_ps[:, c0:c1],
                                        xtT_s[kc][:, t * B:(t + 1) * B],
                                        i2hT_s[kc][:, c0:c1],
                                        start=(kc == 0), stop=False)
                                for rc in range(RCN):
                                    nc.tensor.matmul(
                                        sums_ps[:, c0:c1],
                                        h_ap(t, rc),
                                        h2hT_s[rc][:, c0:c1],
                                        start=False, stop=(rc == RCN - 1))

                            ah_ps = psum.tile([B, H], FP32, tag="ah",
                                              name="ah", bufs=1)
                            for rc2 in range(4):
                                if t == 0:
                                    lhs = AP(zf8[:].tensor, zf8[:].offset,
                                             [list(zf8[:].ap[0]),
                                              [128, 2], [1, 64]])
                                else:
                                    fb = h2f8b[(t - 1) // 2]
                                    lhs = AP(fb[:].tensor,
                                             fb[:].offset + rc2 * 256
                                             + ((t - 1) % 2) * 64,
                                             [list(fb[:].ap[0]),
                                              [128, 2], [1, 64]])
                                rhs = AP(h2attT_s[rc2][:].tensor,
                                         h2attT_s[rc2][:].offset,
                                         [list(h2attT_s[rc2][:].ap[0]),
                                          [H, 2], [1, H]])
                                nc.tensor.matmul(ah_ps[:], lhs, rhs,
                                                 start=(rc2 == 0),
                                                 stop=False,
                                                 perf_mode=PM.DoubleRow)
                            nc.tensor.matmul(ah_ps[:], ones64[:],
                                             h2att_b_s[:], start=False,
                                             stop=True)
                            ah_sb = work.tile([B, H], BF16, tag="ah_sb",
                                              name="ah_sb", bufs=1)
                            nc.scalar.copy(ah_sb[:], ah_ps[:])
                            ahT_ps = psum.tile([128, HCN * 8], FP32,
                                               tag="small", name="ahT_ps",
                                               bufs=1)
                            for hc in range(HCN):
                                nc.tensor.matmul(
                                    ahT_ps[:, hc * 8:(hc + 1) * 8],
                                    ah_sb[:, hc * 128:(hc + 1) * 128],
                                    bsel_s[:], start=True, stop=True)
                            ahT = work.tile([128, HCN * 8], BF16,
                                            tag="ahT_sb", name="ahT_sb",
                                            bufs=1)
                            nc.vector.tensor_copy(ahT[:], ahT_ps[:])

                            e_ps = psum.tile([BMY, L], FP32, tag="small",
                                             name="e_ps", bufs=1)
                            for hcp in range(2):
                                dt2 = work.tile([128, 2 * NBL], FP8,
                                                tag="dt2", name="dt2",
                                                bufs=2)
                                for i in range(2):
                                    hc = 2 * hcp + i
                                    dp = work.tile([128, NBL], BF16,
                                                   tag="dp", name="dp",
                                                   bufs=2)
                                    nc.vector.tensor_tensor(
                                        dp[:].rearrange("p (b l) -> p b l",
                                                        b=BMY),
                                        p_attT[hc][:].rearrange(
                                            "p (b l) -> p b l", b=BMY),
                                        bcast_free(
                                            ahT[:, hc * 8:(hc + 1) * 8], L),
                                        op=ALU.add)
                                    nc.scalar.activation(
                                        dt2[:, i * NBL:(i + 1) * NBL],
                                        dp[:], AF.Tanh)
                                for b in range(BMY):
                                    lhs = AP(alpha_s[hcp][:].tensor,
                                             alpha_s[hcp][:].offset + b * 32,
                                             [list(alpha_s[hcp][:].ap[0]),
                                              [16, 2], [1, BMY]])
                                    rhs = AP(dt2[:].tensor,
                                             dt2[:].offset + b * L,
                                             [list(dt2[:].ap[0]),
                                              [NBL, 2], [1, L]])
                                    nc.tensor.matmul(
                                        e_ps[:], lhs, rhs,
                                        start=(hcp == 0 and b == 0),
                                        stop=(hcp == 1 and b == BMY - 1),
                                        perf_mode=PM.DoubleRow)

                            # e_ps holds 32*e; exp rescales via scale=1/32
                            negm = work.tile([BMY, 1], FP32, tag="negm",
                                             name="negm", bufs=1)
                            nc.vector.tensor_reduce(negm[:], e_ps[:],
                                                    axis=AX.X, op=ALU.max,
                                                    negate=True)
                            negm_s = work.tile([BMY, 1], FP32, tag="negm_s",
                                               name="negm_s", bufs=1)
                            nc.vector.tensor_scalar(negm_s[:], negm[:],
                                                    1.0 / W_SCALE, None,
                                                    op0=ALU.mult)
                            u = work.tile([BMY, L], FP32, tag="u", name="u",
                                          bufs=1)
                            ssum = work.tile([BMY, 1], FP32, tag="ssum",
                                             name="ssum", bufs=1)
                            nc.scalar.activation(u[:], e_ps[:], AF.Exp,
                                                 bias=negm_s[:],
                                                 scale=1.0 / W_SCALE,
                                                 accum_out=ssum[:])
                            rinv = work.tile([BMY, 1], FP32, tag="rinv",
                                             name="rinv", bufs=1)
                            nc.vector.reciprocal(rinv[:], ssum[:])
                            rinv32 = work.tile([BMY, 1], FP32, tag="rinv32",
                                               name="rinv32", bufs=1)
                            nc.vector.tensor_scalar(rinv32[:], rinv[:],
                                                    W_SCALE, None,
                                                    op0=ALU.mult)
                            nc.vector.tensor_scalar(w_f8[:, 0:L], u[:],
                                                    rinv32[:], None,
                                                    op0=ALU.mult)

                            wdr = dpool.tile([BMY, LP], FP8, tag="wdr",
                                             name="wdr")
                            nc.sync.dma_start(out=wdr[:], in_=w_f8[:])
                            for b in range(BMY):
                                nc.sync.dma_start(
                                    out=stat_all[b * 16:(b + 1) * 16,
                                                 b:LP:16],
                                    in_=wdr[b:b + 1, :].rearrange(
                                        "o (g lp) -> (o lp) g", g=NG))

                            # att_res: fp8 DoubleRow, 7 k-tile pairs
                            ar_sb = work.tile([BMY, F], BF16, tag="ar_sb",
                                              name="ar_sb", bufs=1)
                            for fq in range(4):
                                f0 = fq * 512
                                ar_ps = psum.tile([BMY, 512], FP32,
                                                  tag="ar", name="ar_ps",
                                                  bufs=1)
                                for q in range(NG // 2):
                                    lhs = AP(stat_all[:].tensor,
                                             stat_all[:].offset + q * 32,
                                             [list(stat_all[:].ap[0]),
                                              [16, 2], [1, BMY]])
                                    rhs = AP(att_f8_s[:].tensor,
                                             att_f8_s[:].offset
                                             + 2 * q * F + f0,
                                             [list(att_f8_s[:].ap[0]),
                                              [F, 2], [1, 512]])
                                    nc.tensor.matmul(
                                        ar_ps[:], lhs, rhs,
                                        start=(q == 0),
                                        stop=(q == NG // 2 - 1),
                                        perf_mode=PM.DoubleRow)
                                nc.vector.tensor_copy(
                                    ar_sb[:, f0:f0 + 512], ar_ps[:])
                            agA_in = dpool.tile([BMY, F], BF16, tag="agA_in",
                                                name="agA_in")
                            nc.sync.dma_start(out=agA_in[:], in_=ar_sb[:])
                            if no_cc:
                                nc.sync.dma_start(out=agA_out[t][0:BMY, :],
                                                  in_=agA_in[:])
                            else:
                                nc.gpsimd.collective_compute(
                                    "AllGather", ALU.bypass,
                                    replica_groups=RG,
                                    ins=[agA_in.opt()], outs=[agA_out[t][:]])

                            # paired logit for block (t-2)//2 runs in the
                            # AllGather window
                            if t >= 2 and t % 2 == 0:
                                emit_logit_pair((t - 2) // 2)

                            arg_sb = work.tile([B, F], BF16, tag="arg_sb",
                                               name="arg_sb", bufs=1)
                            nc.sync.dma_start(out=arg_sb[:],
                                              in_=agA_out[t][:])
                            arT = work.tile([128, FCN * 64], FP8, tag="arT",
                                            name="arT", bufs=1)
                            for fc in range(FCN):
                                art_ps = psum.tile(
                                    [128, 64], BF16,
                                    tag=("small" if fc % 2 else "ctx"),
                                    name="art_ps", bufs=1)
                                nc.tensor.transpose(
                                    art_ps[:],
                                    arg_sb[:, fc * 128:(fc + 1) * 128],
                                    ident_s[0:B, 0:B])
                                nc.vector.tensor_copy(
                                    arT[:, fc * 64:(fc + 1) * 64],
                                    art_ps[:])

                            # ctx_ps holds 512*ctx (32 from w, 16 from a2c)
                            ctx_ps = psum.tile([B, 256], FP32, tag="ctx",
                                               name="ctx_ps", bufs=1)
                            for fc2 in range(8):
                                lhs = AP(arT[:].tensor,
                                         arT[:].offset + fc2 * 128,
                                         [list(arT[:].ap[0]),
                                          [64, 2], [1, 64]])
                                rhs = AP(a2cT_s[fc2][:].tensor,
                                         a2cT_s[fc2][:].offset,
                                         [list(a2cT_s[fc2][:].ap[0]),
                                          [256, 2], [1, 256]])
                                nc.tensor.matmul(
                                    ctx_ps[:], lhs, rhs, start=(fc2 == 0),
                                    stop=False, perf_mode=PM.DoubleRow)
                            nc.tensor.matmul(ctx_ps[:], ones64[:],
                                             a2c_b_s[:], start=False,
                                             stop=True)

                            sig3 = work.tile([B, 384], FP32, tag="sig3",
                                             name="sig3", bufs=1)
                            nc.scalar.activation(sig3[:], sums_ps[:, 0:384],
                                                 AF.Tanh, scale=0.5)
                            sitr = work.tile([B, 256], FP32, tag="sitr",
                                             name="sitr", bufs=1)
                            nc.scalar.copy(sitr[:], sums_ps[:, 384:640])
                            itr1 = work.tile([B, GC], FP32, tag="itr1",
                                             name="itr1", bufs=1)
                            nc.vector.scalar_tensor_tensor(
                                itr1[:], ctx_ps[:, 0:128], 1.0 / 512.0,
                                sitr[:, 0:128], op0=ALU.mult, op1=ALU.add)
                            itr2 = work.tile([B, GC], FP32, tag="itr2",
                                             name="itr2", bufs=1)
                            nc.vector.scalar_tensor_tensor(
                                itr2[:], ctx_ps[:, 128:256], 1.0 / 512.0,
                                sitr[:, 128:256], op0=ALU.mult, op1=ALU.add)
                            g_t = work.tile([B, GC], FP32, tag="g_t",
                                            name="g_t", bufs=1)
                            nc.vector.tensor_tensor(g_t[:], itr1[:],
                                                    itr2[:], op=ALU.max)
                            a_t = work.tile([B, GC], FP32, tag="a_t",
                                            name="a_t", bufs=1)
                            nc.vector.scalar_tensor_tensor(
                                a_t[:], sig3[:, 128:256], 1.0, c_st[:],
                                op0=ALU.add, op1=ALU.mult)
                            b_t = work.tile([B, GC], FP32, tag="b_t",
                                            name="b_t", bufs=1)
                            nc.vector.scalar_tensor_tensor(
                                b_t[:], sig3[:, 0:128], 1.0, g_t[:],
                                op0=ALU.add, op1=ALU.mult)
                            nc2_t = work.tile([B, GC], FP32, tag="nc2",
                                              name="nc2", bufs=1)
                            nc.vector.tensor_tensor(nc2_t[:], a_t[:],
                                                    b_t[:], op=ALU.add)
                            nc.vector.tensor_scalar(c_st[:], nc2_t[:], 0.5,
                                                    None, op0=ALU.mult)
                            tnc = work.tile([B, GC], FP32, tag="tnc",
                                            name="tnc", bufs=1)
                            nc.scalar.activation(tnc[:], nc2_t[:], AF.Tanh,
                                                 scale=0.5)
                            nh2 = work.tile([B, GC], BF16, tag="nh2",
                                            name="nh2", bufs=1)
                            nc.vector.scalar_tensor_tensor(
                                nh2[:], sig3[:, 256:384], 1.0, tnc[:],
                                op0=ALU.add, op1=ALU.mult)

                            nhT_ps = psum.tile([GC, B], BF16, tag="small",
                                               name="nhT_ps", bufs=1)
                            nc.tensor.transpose(nhT_ps[:], nh2[:],
                                                ident_s[0:B, 0:B])
                            nhT_sb = work.tile([GC, B], BF16, tag="nhT_sb",
                                               name="nhT_sb", bufs=1)
                            nc.vector.tensor_copy(nhT_sb[:], nhT_ps[:])
                            agH_in = dpool.tile([GC, B], BF16, tag="agH_in",
                                                name="agH_in")
                            nc.sync.dma_start(out=agH_in[:], in_=nhT_sb[:])
                            if no_cc:
                                nc.sync.dma_start(out=agH_out[t][0:GC, :],
                                                  in_=agH_in[:])
                            else:
                                nc.gpsimd.collective_compute(
                                    "AllGather", ALU.bypass,
                                    replica_groups=RG,
                                    ins=[agH_in.opt()], outs=[agH_out[t][:]])
                            # h_{t+1} -> block t//2, half t%2
                            if t % 2 == 0:
                                h2b[t // 2] = hpool.tile(
                                    [128, RCN * 128], BF16, tag="h2",
                                    name=f"h2_{t // 2}")
                                h2f8b[t // 2] = hpool.tile(
                                    [128, RCN * 128], FP8, tag="h2f8",
                                    name=f"h2f8_{t // 2}")
                            blk = h2b[t // 2]
                            dst = AP(blk[:].tensor,
                                     blk[:].offset + (t % 2) * 64,
                                     [list(blk[:].ap[0]), [128, RCN],
                                      [1, 64]])
                            nc.sync.dma_start(
                                out=dst,
                                in_=agH_out[t][:].rearrange(
                                    "(rc rl) b -> rl rc b", rc=RCN))
                            fblk = h2f8b[t // 2]
                            fsrc = AP(blk[:].tensor,
                                      blk[:].offset + (t % 2) * 64,
                                      [list(blk[:].ap[0]), [128, RCN],
                                       [1, 64]])
                            fdst = AP(fblk[:].tensor,
                                      fblk[:].offset + (t % 2) * 64,
                                      [list(fblk[:].ap[0]), [128, RCN],
                                       [1, 64]])
                            nc.vector.tensor_copy(fdst, fsrc)

                            if t == 0:
                                probe("ah0", ah_sb[:], [B, H], BF16)
                                probe("ahT0", ahT[:], [128, HCN * 8], BF16)
                                probe("u0", u[:], [BMY, L], FP32)
                                probe("statall0", stat_all[:],
                                      [128, LP], FP8)
                                probe("ar0", ar_sb[:], [BMY, F], BF16)
                                probe("arT0", arT[:], [128, FCN * 64], BF16)
                                probe("nh20", nh2[:], [B, GC], BF16)
                                probe("agH0", agH_out[0][:], [R, B], BF16)

                        # remaining pairs + stats
                        emit_logit_pair(NPAIR - 1)
                        emit_pair_stats(NPAIR - 1, work)
                        probe("lgb0", lgb[0][:], [128, VP], BF16)

                    # ---------- phase 2 ----------
                    with tc.tile_pool(name=f"p2_{rep}", bufs=2) as p2:
                        agS_in = dpool.tile([128, 2 * NT], FP32,
                                            tag="agS_in", name="agS_in")
                        nc.sync.dma_start(out=agS_in[:, 0:NT],
                                          in_=negm_all[:])
                        nc.sync.dma_start(out=agS_in[:, NT:2 * NT],
                                          in_=s_all[:])
                        if no_cc:
                            nc.sync.dma_start(out=agS_out[0:128, :],
                                              in_=agS_in[:])
                        else:
                            nc.gpsimd.collective_compute(
                                "AllGather", ALU.bypass, replica_groups=RG,
                                ins=[agS_in.opt()], outs=[agS_out[:]])
                        statg = p2.tile([128, NC * 2 * NT], FP32,
                                        tag="statg", name="statg", bufs=1)
                        nc.sync.dma_start(
                            out=statg[:].rearrange("p (r s) -> p r s", r=NC),
                            in_=agS_out[:].rearrange("(r p) s -> p r s",
                                                     r=NC))
                        sview = statg[:].rearrange("p (r s) -> p s r", r=NC)
                        negM = p2.tile([128, NT], FP32, tag="negM",
                                       name="negM", bufs=1)
                        nc.vector.tensor_reduce(negM[:], sview[:, 0:NT, :],
                                                axis=AX.X, op=ALU.min)
                        earg = p2.tile([128, NT * NC], FP32, tag="earg",
                                       name="earg", bufs=1)
                        nc.vector.tensor_tensor(
                            earg[:].rearrange("p (s r) -> p s r", r=NC),
                            bcast_free(negM[:], NC), sview[:, 0:NT, :],
                            op=ALU.subtract)
                        em = p2.tile([128, NT * NC], FP32, tag="em",
                                     name="em", bufs=1)
                        nc.scalar.activation(em[:], earg[:], AF.Exp)
                        sexp = p2.tile([128, NT * NC], FP32, tag="sexp",
                                       name="sexp", bufs=1)
                        nc.vector.tensor_tensor(
                            sexp[:].rearrange("p (s r) -> p s r", r=NC),
                            em[:].rearrange("p (s r) -> p s r", r=NC),
                            sview[:, NT:2 * NT, :], op=ALU.mult)
                        S_t = p2.tile([128, NT], FP32, tag="S_t",
                                      name="S_t", bufs=1)
                        nc.vector.tensor_reduce(
                            S_t[:],
                            sexp[:].rearrange("p (s r) -> p s r", r=NC),
                            axis=AX.X, op=ALU.add)
                        lnS = p2.tile([128, NT], FP32, tag="lnS",
                                      name="lnS", bufs=1)
                        nc.scalar.activation(lnS[:], S_t[:], AF.Ln)
                        logZ = p2.tile([128, NT], FP32, tag="logZ",
                                       name="logZ", bufs=1)
                        nc.vector.scalar_tensor_tensor(
                            logZ[:], negM[:], -1.0, lnS[:], op0=ALU.mult,
                            op1=ALU.add)
                        probe("logZ", logZ[:], [128, NT], FP32)
                        for tt in range(NT):
                            lp_t = p2.tile([128, VP], FP32, tag="p2lp",
                                           name="p2lp")
                            nc.vector.tensor_scalar(
                                lp_t[:], lgb[tt][:], logZ[:, tt:tt + 1],
                                None, op0=ALU.subtract)
                            nc.sync.dma_start(
                                out=out_d[tt * 128:(tt + 1) * 128, :],
                                in_=lp_t[:])

                for rep in range(reps):
                    emit_rep(rep)

    nc.compile()
    return nc, sorted(probes)


_NC_CACHE = {}


def kernel(**inputs):
    """Full-input entry point: returns logp [B, T, V1] float32."""
    from concourse.bass_utils import run_bass_kernel_spmd
    in_maps = host_prep(inputs)
    if "nc" not in _NC_CACHE:
        _NC_CACHE["nc"], _ = build(T, (), reps=1)
    nc = _NC_CACHE["nc"]
    res = run_bass_kernel_spmd(nc, in_maps, list(range(NC)))
    outs = [res.results[c]["logp"] for c in range(NC)]
    full = np.concatenate(outs, axis=1)[:, :V1]          # [T*B, V1]
    logp = full.reshape(T, B, V1).transpose(1, 0, 2)
    return np.ascontiguousarray(logp.astype(np.float32))


# revision 30
# speedup vs baseline: 1.0416x; 1.0416x over previous
"""Attention-LSTM captioning model on 8 trn2 cores (8-way tensor parallel).

Gate/itr/vocab output dims sharded across cores (full B=64 per core);
attention batch-sharded (8 batches/core, selected via per-core one-hot bsel
data, since the SPMD program is identical on every core). Activations are
transposed [feature, batch]. Per step: AllGather(att_res), AllGather(nh
chunk). Sigmoid(x) is computed as (tanh(x/2)+1)/2 so phase 1 only needs the
{tanh, exp} ACT table; the hidden state is stored as 2*h with h-consuming
weights pre-halved on the host.

v2: att_res matmul runs in fp8e4 DoubleRow mode (w scaled x32, a2c
pre-divided); the logit matmul is computed per step-pair with M=128 (both
steps' h in one stationary tile), scheduled one pair late so it lands in
the att_res AllGather window; logits stay in SBUF as bf16 and log_softmax
stats are folded into phase 1, so phase 2 is just one stats AllGather and
the final normalize (no DRAM scratch roundtrip).
"""
import numpy as np
import ml_dtypes

import concourse.bacc as bacc
import concourse.mybir as mybir
import concourse.tile as tile
from concourse.ap import AP
from concourse.bass_utils import run_bass_kernel_spmd

BF16_NP = ml_dtypes.bfloat16
FP8_NP = ml_dtypes.float8_e4m3
FP32 = mybir.dt.float32
BF16 = mybir.dt.bfloat16
FP8 = mybir.dt.float8e4
AF = mybir.ActivationFunctionType
ALU = mybir.AluOpType
AX = mybir.AxisListType
PM = mybir.MatmulPerfMode

B, T, R, H, F, E, L, V1 = 64, 20, 1024, 512, 2048, 300, 196, 12001
NC = 8
BMY = B // NC
GC = R // NC              # 128
NGATE = 5 * GC            # 640
VP = 1504
NG = 14                   # l-groups of 16 (224 >= L), even for fp8 pairs
LP = NG * 16              # 224
W_SCALE = 32.0
EP = 384
HCN = H // 128            # 4
FCN = F // 128            # 16
RCN = R // 128            # 8
NBL = BMY * L             # 1568


def _bf(x):
    return np.ascontiguousarray(np.asarray(x, dtype=np.float32)).astype(BF16_NP)


def _f8(x):
    return np.ascontiguousarray(np.asarray(x, dtype=np.float32)).astype(FP8_NP)


def bcast_free(ap, n):
    """Append a step-0 free dim of size n to an AP (broadcast)."""
    return AP(ap.tensor, ap.offset, list(ap.ap) + [[0, n]])


def host_prep(inputs):
    seq = np.asarray(inputs["seq"])
    att = np.asarray(inputs["att_feats"], dtype=np.float32)
    embed_w = np.asarray(inputs["embed_w"], dtype=np.float32)
    ctx2att_w = np.asarray(inputs["ctx2att_w"], dtype=np.float32)
    ctx2att_b = np.asarray(inputs["ctx2att_b"], dtype=np.float32)
    h2att_w = np.asarray(inputs["h2att_w"], dtype=np.float32)
    h2att_b = np.asarray(inputs["h2att_b"], dtype=np.float32)
    alpha_w = np.asarray(inputs["alpha_w"], dtype=np.float32)
    i2h_w = np.asarray(inputs["i2h_w"], dtype=np.float32)
    i2h_b = np.asarray(inputs["i2h_b"], dtype=np.float32)
    h2h_w = np.asarray(inputs["h2h_w"], dtype=np.float32)
    h2h_b = np.asarray(inputs["h2h_b"], dtype=np.float32)
    a2c_w = np.asarray(inputs["a2c_w"], dtype=np.float32)
    a2c_b = np.asarray(inputs["a2c_b"], dtype=np.float32)
    logit_w = np.asarray(inputs["logit_w"], dtype=np.float32)
    logit_b = np.asarray(inputs["logit_b"], dtype=np.float32)

    xt = embed_w[seq]                                    # [B, T, E]
    xtT = np.zeros((EP, T * B), dtype=np.float32)
    xtT[:E] = xt.transpose(2, 1, 0).reshape(E, T * B)
    xtT[E] = 1.0
    xtT = _bf(xtT)
    bias_gate = i2h_b + h2h_b

    in_maps = []
    for c in range(NC):
        m = {"xtT": xtT}
        grows = np.concatenate([np.arange(gg * R + c * GC, gg * R + (c + 1) * GC)
                                for gg in range(5)])
        i2hT = np.zeros((EP, NGATE), dtype=np.float32)
        i2hT[:E] = i2h_w[grows, :].T
        i2hT[E] = bias_gate[grows]
        m["i2hT"] = _bf(i2hT)
        m["h2hT"] = _bf(h2h_w[grows, :].T * 0.5)
        ht8 = (h2att_w.T * 0.5).reshape(4, 2, 128, H)
        m["h2attT_f8"] = _f8(ht8.transpose(0, 2, 1, 3).reshape(512, 2 * H))
        m["h2att_bias"] = _bf(h2att_b[None, :])
        m["ctxT"] = _bf(ctx2att_w.T)
        m["ctx_bias"] = _bf(ctx2att_b[None, :])
        amy = att[c * BMY:(c + 1) * BMY]                 # [8, L, F]
        m["attT_f8"] = _f8(amy.transpose(2, 0, 1).reshape(F, NBL))
        # ctx2att in fp8 k-tile-pair layout: [pair*128+p, i*H+h] =
        # ctx2att_w.T[pair*256+i*128+p, h]
        cT = ctx2att_w.T.reshape(8, 2, 128, H)           # [pair, i, p, h]
        m["ctxT_f8"] = _f8(cT.transpose(0, 2, 1, 3).reshape(1024, 2 * H))
        # fp8 att for the att_res matmul: part p=(b*16+lg), col=(g*F+f),
        # value att[b, g*16+lg, f]
        apad = np.zeros((BMY, LP, F), dtype=np.float32)
        apad[:, :L] = amy
        m["att_f8"] = _f8(apad.reshape(BMY, NG, 16, F).transpose(0, 2, 1, 3)
                          .reshape(128, NG * F))
        # alpha one-hot diag, fp8 k-pair layout per hc-pair:
        # [hcp*128+p, b*32+i*16+m] = 32*alpha[(2hcp+i)*128+p] iff m==b
        ac = np.zeros((2, 128, BMY, 2, 16), dtype=np.float32)
        for hcp in range(2):
            for i in range(2):
                for b in range(BMY):
                    ac[hcp, :, b, i, b] = \
                        W_SCALE * alpha_w[0, (2 * hcp + i) * 128:
                                          (2 * hcp + i + 1) * 128]
        m["alpha_f8"] = _f8(ac.reshape(256, BMY * 32))
        arows = np.concatenate([np.arange(c * GC, (c + 1) * GC),
                                np.arange(R + c * GC, R + (c + 1) * GC)])
        # a2c in fp8 k-pair layout, scaled x16 (itr add divides by 512)
        a2cp = (a2c_w[arows, :].T * 16.0).reshape(8, 2, 128, 256)
        m["a2cT_f8"] = _f8(a2cp.transpose(0, 2, 1, 3).reshape(1024, 512))
        m["a2c_bias"] = _bf(a2c_b[arows][None, :] * 512.0)
        vrows = np.arange(c * VP, (c + 1) * VP)
        lw = np.zeros((R, VP), dtype=np.float32)
        lb = np.full((1, VP), -1e30, dtype=np.float32)
        valid = vrows < V1
        lw[:, valid] = logit_w[vrows[valid], :].T * 0.5
        lb[0, valid] = logit_b[vrows[valid]]
        # fp8 k-pair layout, x8 for fp8 range (bias-add divides by 8)
        lwp = (lw * 8.0).reshape(4, 2, 128, VP)
        m["logitT_f8"] = _f8(lwp.transpose(0, 2, 1, 3).reshape(512, 2 * VP))
        m["logit_bias"] = lb
        m["ident"] = _bf(np.eye(128))
        bsel = np.zeros((B, BMY), dtype=np.float32)
        for j in range(BMY):
            bsel[c * BMY + j, j] = 1.0
        m["bsel"] = _bf(bsel)
        in_maps.append(m)
    return in_maps


def build(t_steps=T, probes=(), reps=1, no_cc=False):
    assert t_steps % 2 == 0
    nc = bacc.Bacc("TRN2", target_bir_lowering=False, debug=False,
                   num_devices=NC)
    probes = set(probes)
    NT = t_steps * B // 128
    NPAIR = t_steps // 2
    RG = [list(range(NC))]

    def din(name, shape, dt=BF16):
        return nc.dram_tensor(name, shape, dt, kind="ExternalInput")

    xtT_d = din("xtT", [EP, T * B])
    i2hT_d = din("i2hT", [EP, NGATE])
    h2hT_d = din("h2hT", [R, NGATE])
    h2attT_d = din("h2attT_f8", [512, 2 * H], FP8)
    h2att_b_d = din("h2att_bias", [1, H])
    ctxT_d = din("ctxT_f8", [1024, 2 * H], FP8)
    ctx_b_d = din("ctx_bias", [1, H])
    attT_d = din("attT_f8", [F, NBL], FP8)
    att_f8_d = din("att_f8", [128, NG * F], FP8)
    alpha_d = din("alpha_f8", [256, BMY * 32], FP8)
    a2cT_d = din("a2cT_f8", [1024, 512], FP8)
    a2c_b_d = din("a2c_bias", [1, 256])
    logitT_d = din("logitT_f8", [512, 2 * VP], FP8)
    logit_b_d = din("logit_bias", [1, VP], FP32)
    ident_d = din("ident", [128, 128])
    bsel_d = din("bsel", [B, BMY])

    out_d = nc.dram_tensor("logp", [t_steps * B, VP], FP32,
                           kind="ExternalOutput")
    agA_out_r = [[nc.dram_tensor(f"agA_out_{rp}_{t}", [B, F], BF16,
                                 addr_space="Shared") for t in range(t_steps)]
                 for rp in range(reps)]
    agH_out_r = [[nc.dram_tensor(f"agH_out_{rp}_{t}", [R, B], BF16,
                                 addr_space="Shared") for t in range(t_steps)]
                 for rp in range(reps)]
    agS_out_r = [nc.dram_tensor(f"agS_out_{rp}", [NC * 128, 2 * NT], FP32,
                                addr_space="Shared") for rp in range(reps)]

    with tile.TileContext(nc) as tc:
        with (
            tc.tile_pool(name="wpool", bufs=1) as wpool,
            tc.tile_pool(name="hpool", bufs=3) as hpool,
            tc.tile_pool(name="psum", bufs=1, space="PSUM") as psum,
            tc.tile_pool(name="dram", bufs=4, space="DRAM") as dpool,
        ):
            def probe_(name, src_ap, shape, dt):
                pd = nc.dram_tensor(f"probe_{name}", list(shape), dt,
                                    kind="ExternalOutput")
                nc.sync.dma_start(out=pd[:], in_=src_ap)

            def load_chunks(pool, dram, cols, n, tag, dt=BF16):
                ts = []
                for i in range(n):
                    t_ = pool.tile([128, cols], dt, tag=f"{tag}{i}",
                                   name=f"{tag}{i}")
                    nc.sync.dma_start(out=t_[:],
                                      in_=dram[i * 128:(i + 1) * 128, :])
                    ts.append(t_)
                return ts

            logitT_s = load_chunks(wpool, logitT_d, 2 * VP, 4, "logitT", FP8)
            logit_b_s = wpool.tile([128, VP], FP32, tag="logitb",
                                   name="logitb")
            _lb_src = AP(logit_b_d[:].tensor, logit_b_d[:].offset,
                         [[0, 128], [1, VP]])
            nc.sync.dma_start(out=logit_b_s[:], in_=_lb_src)
            ident_s = wpool.tile([128, 128], BF16, tag="ident", name="ident")
            nc.sync.dma_start(out=ident_s[:], in_=ident_d[:])
            ones64 = wpool.tile([1, B], BF16, tag="ones64", name="ones64")
            nc.vector.memset(ones64[:], 1.0)
            negm_all = wpool.tile([128, NT], FP32, tag="negm_all",
                                  name="negm_all")
            s_all = wpool.tile([128, NT], FP32, tag="s_all", name="s_all")
            # bf16 logits resident in SBUF, one tile per step-pair
            lgb = [wpool.tile([128, VP], BF16, tag=f"lgb{k}", name=f"lgb{k}")
                   for k in range(NPAIR)]

            with tc.tile_pool(name="w1pool", bufs=1) as w1pool:
                xtT_s = load_chunks(w1pool, xtT_d, T * B, 3, "xtT")
                i2hT_s = load_chunks(w1pool, i2hT_d, NGATE, 3, "i2hT")
                h2hT_s = load_chunks(w1pool, h2hT_d, NGATE, RCN, "h2hT")
                h2attT_s = load_chunks(w1pool, h2attT_d, 2 * H, 4,
                                       "h2attT", FP8)
                att_f8_s = w1pool.tile([128, NG * F], FP8, tag="attf8",
                                       name="attf8")
                nc.sync.dma_start(out=att_f8_s[:], in_=att_f8_d[:])
                alpha_s = load_chunks(w1pool, alpha_d, BMY * 32, 2,
                                      "alpha", FP8)
                a2cT_s = load_chunks(w1pool, a2cT_d, 512, 8, "a2cT", FP8)
                bsel_s = w1pool.tile([B, BMY], BF16, tag="bsel", name="bsel")
                nc.sync.dma_start(out=bsel_s[:], in_=bsel_d[:])
                h2att_b_s = w1pool.tile([1, H], BF16, tag="h2attb",
                                        name="h2attb")
                nc.sync.dma_start(out=h2att_b_s[:], in_=h2att_b_d[:])
                ctx_b_s = w1pool.tile([1, H], BF16, tag="ctxb", name="ctxb")
                nc.sync.dma_start(out=ctx_b_s[:], in_=ctx_b_d[:])
                a2c_b_s = w1pool.tile([1, 256], BF16, tag="a2cb", name="a2cb")
                nc.sync.dma_start(out=a2c_b_s[:], in_=a2c_b_d[:])
                onesNBL = w1pool.tile([1, NBL], BF16, tag="onesNBL",
                                      name="onesNBL")
                nc.vector.memset(onesNBL[:], 1.0)
                p_attT = [w1pool.tile([128, NBL], BF16, tag=f"pattT{hc}",
                                      name=f"pattT{hc}")
                          for hc in range(HCN)]
                stat_all = w1pool.tile([128, LP], FP8, tag="stat_all",
                                       name="stat_all")
                nc.vector.memset(stat_all[:], 0.0)
                w_f8 = w1pool.tile([BMY, LP], FP8, tag="w_f8", name="w_f8")
                nc.vector.memset(w_f8[:], 0.0)
                zh = w1pool.tile([128, 64], BF16, tag="zh", name="zh")
                nc.vector.memset(zh[:], 0.0)
                zf8 = w1pool.tile([128, 384], FP8, tag="zf8", name="zf8")
                nc.vector.memset(zf8[:], 0.0)
                c_st = w1pool.tile([B, GC], FP32, tag="c_st", name="c_st")

                def emit_rep(rep):
                    agA_out = agA_out_r[rep]
                    agH_out = agH_out_r[rep]
                    agS_out = agS_out_r[rep]

                    def probe(name, src_ap, shape, dt):
                        if rep == 0 and name in probes:
                            probe_(name, src_ap, shape, dt)

                    nc.vector.memset(c_st[:], 0.0)
                    # h2 blocks: [128, rc(8) x half(2) x b(64)]; block k
                    # holds h_{2k+1} (half 0) and h_{2k+2} (half 1), as 2*h.
                    # h2f8 blocks mirror them in fp8 for ah/logit matmuls.
                    h2b = [None] * NPAIR
                    h2f8b = [None] * NPAIR

                    def h_ap(j, rc):
                        """lhsT slice [128, 64] for h_j, R-chunk rc."""
                        if j == 0:
                            return zh[:]
                        blk = h2b[(j - 1) // 2]
                        c0 = rc * 128 + ((j - 1) % 2) * 64
                        return blk[:, c0:c0 + 64]

                    # ---------- phase 0 ----------
                    with (
                        tc.tile_pool(name=f"ctxpool{rep}", bufs=1) as ctxpool,
                        tc.tile_pool(name=f"stream{rep}", bufs=3) as stream,
                    ):
                        ctxT_s = load_chunks(ctxpool, ctxT_d, 2 * H, 8,
                                             "ctxT", FP8)
                        QW = 392
                        for q in range(4):
                            n0 = q * QW
                            _pa_tags = ["sums", "ah", "ar", "lg"]
                            pa_ps = [psum.tile([128, QW], FP32,
                                               tag=_pa_tags[hc],
                                               name=f"pa{hc}",
                                               bufs=(2 if hc == 3 else 1))
                                     for hc in range(HCN)]
                            for fc2 in range(8):
                                at = stream.tile([128, 2 * QW], FP8,
                                                 tag="attTq", name="attTq")
                                nc.sync.dma_start(
                                    out=at[:].rearrange(
                                        "p (two n) -> p two n", two=2),
                                    in_=attT_d[fc2 * 256:(fc2 + 1) * 256,
                                               n0:n0 + QW].rearrange(
                                        "(two p) n -> p two n", two=2))
                                for hc in range(HCN):
                                    lhs = AP(ctxT_s[fc2][:].tensor,
                                             ctxT_s[fc2][:].offset
                                             + hc * 128,
                                             [list(ctxT_s[fc2][:].ap[0]),
                                              [H, 2], [1, 128]])
                                    nc.tensor.matmul(
                                        pa_ps[hc][:], lhs,
                                        at[:].rearrange(
                                            "p (two n) -> p two n", two=2),
                                        start=(fc2 == 0), stop=False,
                                        perf_mode=PM.DoubleRow)
                            for hc in range(HCN):
                                nc.tensor.matmul(
                                    pa_ps[hc][:],
                                    ctx_b_s[:, hc * 128:(hc + 1) * 128],
                                    onesNBL[:, n0:n0 + QW], start=False,
                                    stop=True)
                                nc.vector.tensor_copy(
                                    p_attT[hc][:, n0:n0 + QW], pa_ps[hc][:])
                    probe("p_attT0", p_attT[0][:], [128, NBL], BF16)

                    def emit_logit_pair(k):
                        """Logit matmul for step pair k from h2b[k] (M=128),
                        bias-add into lgb[k] (bf16)."""
                        fblk = h2f8b[k]
                        for ci, c0 in enumerate((0, 512, 1024)):
                            c1 = min(VP, c0 + 512)
                            lg_ps = psum.tile([128, 512], FP32, tag="lg",
                                              name="lg_ps", bufs=2)
                            for rc2 in range(4):
                                lhs = AP(fblk[:].tensor,
                                         fblk[:].offset + rc2 * 256,
                                         [list(fblk[:].ap[0]),
                                          [128, 2], [1, 128]])
                                rhs = AP(logitT_s[rc2][:].tensor,
                                         logitT_s[rc2][:].offset + c0,
                                         [list(logitT_s[rc2][:].ap[0]),
                                          [VP, 2], [1, c1 - c0]])
                                nc.tensor.matmul(
                                    lg_ps[:, 0:c1 - c0], lhs, rhs,
                                    start=(rc2 == 0), stop=(rc2 == 3),
                                    perf_mode=PM.DoubleRow)
                            nc.vector.scalar_tensor_tensor(
                                lgb[k][:, c0:c1], lg_ps[:, 0:c1 - c0],
                                1.0 / 8.0, logit_b_s[:, c0:c1],
                                op0=ALU.mult, op1=ALU.add)

                    def emit_pair_stats(k, work):
                        """Softmax stats for pair k (reads lgb[k], SBUF)."""
                        nc.vector.tensor_reduce(
                            negm_all[:, k:k + 1], lgb[k][:], axis=AX.X,
                            op=ALU.max, negate=True)
                        junk = work.tile([128, VP], BF16, tag="p2junk",
                                         name="p2junk", bufs=2)
                        nc.scalar.activation(
                            junk[:], lgb[k][:], AF.Exp,
                            bias=negm_all[:, k:k + 1],
                            accum_out=s_all[:, k:k + 1])

                    # ---------- phase 1 ----------
                    with tc.tile_pool(name=f"work1_{rep}", bufs=1) as work:
                        for t in range(t_steps):
                            # stats for the pair computed two steps ago run
                            # in this step's sums/ah window (DVE+ACT idle)
                            if t >= 3 and t % 2 == 1:
                                emit_pair_stats((t - 3) // 2, work)
                            sums_ps = psum.tile([B, NGATE], FP32, tag="sums",
                                                name="sums", bufs=1)
                            for c0 in (0, 512):
                                c1 = min(NGATE, c0 + 512)
                                for kc in range(3):
                                    nc.tensor.matmul(
                                        sums_ps[:, c0:c1],
                                        xtT_s[kc][:, t * B:(t + 1) * B],
                                        i2hT_s[kc][:, c0:c1],
                                        start=(kc == 0), stop=False)
                                for rc in range(RCN):
                                    nc.tensor.matmul(
                                        sums_ps[:, c0:c1],
                                        h_ap(t, rc),
                                        h2hT_s[rc][:, c0:c1],
                                        start=False, stop=(rc == RCN - 1))

                            ah_ps = psum.tile([B, H], FP32, tag="ah",
                                              name="ah", bufs=1)
                            for rc2 in range(4):
                                if t == 0:
                                    lhs = AP(zf8[:].tensor, zf8[:].offset,
                                             [list(zf8[:].ap[0]),
                                              [128, 2], [1, 64]])
                                else:
                                    fb = h2f8b[(t - 1) // 2]
                                    lhs = AP(fb[:].tensor,
                                             fb[:].offset + rc2 * 256
                                             + ((t - 1) % 2) * 64,
                                             [list(fb[:].ap[0]),
                                              [128, 2], [1, 64]])
                                rhs = AP(h2attT_s[rc2][:].tensor,
                                         h2attT_s[rc2][:].offset,
                                         [list(h2attT_s[rc2][:].ap[0]),
                                          [H, 2], [1, H]])
                                nc.tensor.matmul(ah_ps[:], lhs, rhs,
                                                 start=(rc2 == 0),
                                                 stop=False,
                                                 perf_mode=PM.DoubleRow)
                            nc.tensor.matmul(ah_ps[:], ones64[:],
                                             h2att_b_s[:], start=False,
                                             stop=True)
                            ah_sb = work.tile([B, H], BF16, tag="ah_sb",
                                              name="ah_sb", bufs=1)
                            nc.scalar.copy(ah_sb[:], ah_ps[:])
                            ahT_ps = psum.tile([128, HCN * 8], FP32,
                                               tag="small", name="ahT_ps",
                                               bufs=1)
                            for hc in range(HCN):
                                nc.tensor.matmul(
                                    ahT_ps[:, hc * 8:(hc + 1) * 8],
                                    ah_sb[:, hc * 128:(hc + 1) * 128],
                                    bsel_s[:], start=True, stop=True)
                            ahT = work.tile([128, HCN * 8], BF16,
                                            tag="ahT_sb", name="ahT_sb",
                                            bufs=1)
                            nc.vector.tensor_copy(ahT[:], ahT_ps[:])

                            e_ps = psum.tile([BMY, L], FP32, tag="small",
                                             name="e_ps", bufs=1)
                            for hcp in range(2):
                                dt2 = work.tile([128, 2 * NBL], FP8,
                                                tag="dt2", name="dt2",
                                                bufs=2)
                                for i in range(2):
                                    hc = 2 * hcp + i
                                    dp = work.tile([128, NBL], BF16,
                                                   tag="dp", name="dp",
                                                   bufs=2)
                                    eng = nc.vector if i == 0 else nc.gpsimd
                                    eng.tensor_tensor(
                                        dp[:].rearrange("p (b l) -> p b l",
                                                        b=BMY),
                                        p_attT[hc][:].rearrange(
                                            "p (b l) -> p b l", b=BMY),
                                        bcast_free(
                                            ahT[:, hc * 8:(hc + 1) * 8], L),
                                        op=ALU.add)
                                    nc.scalar.activation(
                                        dt2[:, i * NBL:(i + 1) * NBL],
                                        dp[:], AF.Tanh)
                                for b in range(BMY):
                                    lhs = AP(alpha_s[hcp][:].tensor,
                                             alpha_s[hcp][:].offset + b * 32,
                                             [list(alpha_s[hcp][:].ap[0]),
                                              [16, 2], [1, BMY]])
                                    rhs = AP(dt2[:].tensor,
                                             dt2[:].offset + b * L,
                                             [list(dt2[:].ap[0]),
                                              [NBL, 2], [1, L]])
                                    nc.tensor.matmul(
                                        e_ps[:], lhs, rhs,
                                        start=(hcp == 0 and b == 0),
                                        stop=(hcp == 1 and b == BMY - 1),
                                        perf_mode=PM.DoubleRow)

                            # e_ps holds 32*e; exp rescales via scale=1/32
                            negm = work.tile([BMY, 1], FP32, tag="negm",
                                             name="negm", bufs=1)
                            nc.vector.tensor_reduce(negm[:], e_ps[:],
                                                    axis=AX.X, op=ALU.max,
                                                    negate=True)
                            negm_s = work.tile([BMY, 1], FP32, tag="negm_s",
                                               name="negm_s", bufs=1)
                            nc.vector.tensor_scalar(negm_s[:], negm[:],
                                                    1.0 / W_SCALE, None,
                                                    op0=ALU.mult)
                            u = work.tile([BMY, L], FP32, tag="u", name="u",
                                          bufs=1)
                            ssum = work.tile([BMY, 1], FP32, tag="ssum",
                                             name="ssum", bufs=1)
                            nc.scalar.activation(u[:], e_ps[:], AF.Exp,
                                                 bias=negm_s[:],
                                                 scale=1.0 / W_SCALE,
                                                 accum_out=ssum[:])
                            rinv = work.tile([BMY, 1], FP32, tag="rinv",
                                             name="rinv", bufs=1)
                            nc.vector.reciprocal(rinv[:], ssum[:])
                            rinv32 = work.tile([BMY, 1], FP32, tag="rinv32",
                                               name="rinv32", bufs=1)
                            nc.vector.tensor_scalar(rinv32[:], rinv[:],
                                                    W_SCALE, None,
                                                    op0=ALU.mult)
                            nc.vector.tensor_scalar(w_f8[:, 0:L], u[:],
                                                    rinv32[:], None,
                                                    op0=ALU.mult)

                            wdr = dpool.tile([BMY, LP], FP8, tag="wdr",
                                             name="wdr")
                            nc.sync.dma_start(out=wdr[:], in_=w_f8[:])
                            for b in range(BMY):
                                nc.sync.dma_start(
                                    out=stat_all[b * 16:(b + 1) * 16,
                                                 b:LP:16],
                                    in_=wdr[b:b + 1, :].rearrange(
                                        "o (g lp) -> (o lp) g", g=NG))

                            # att_res: fp8 DoubleRow, 7 k-tile pairs
                            ar_sb = work.tile([BMY, F], BF16, tag="ar_sb",
                                              name="ar_sb", bufs=1)
                            for fq in range(4):
                                f0 = fq * 512
                                ar_ps = psum.tile([BMY, 512], FP32,
                                                  tag="ar", name="ar_ps",
                                                  bufs=1)
                                for q in range(NG // 2):
                                    lhs = AP(stat_all[:].tensor,
                                             stat_all[:].offset + q * 32,
                                             [list(stat_all[:].ap[0]),
                                              [16, 2], [1, BMY]])
                                    rhs = AP(att_f8_s[:].tensor,
                                             att_f8_s[:].offset
                                             + 2 * q * F + f0,
                                             [list(att_f8_s[:].ap[0]),
                                              [F, 2], [1, 512]])
                                    nc.tensor.matmul(
                                        ar_ps[:], lhs, rhs,
                                        start=(q == 0),
                                        stop=(q == NG // 2 - 1),
                                        perf_mode=PM.DoubleRow)
                                nc.vector.tensor_copy(
                                    ar_sb[:, f0:f0 + 512], ar_ps[:])
                            agA_in = dpool.tile([BMY, F], BF16, tag="agA_in",
                                                name="agA_in")
                            nc.sync.dma_start(out=agA_in[:], in_=ar_sb[:])
                            if no_cc:
                                nc.sync.dma_start(out=agA_out[t][0:BMY, :],
                                                  in_=agA_in[:])
                            else:
                                nc.gpsimd.collective_compute(
                                    "AllGather", ALU.bypass,
                                    replica_groups=RG,
                                    ins=[agA_in.opt()], outs=[agA_out[t][:]])

                            # paired logit for block (t-2)//2 runs in the
                            # AllGather window
                            if t >= 2 and t % 2 == 0:
                                emit_logit_pair((t - 2) // 2)

                            arg_sb = work.tile([B, F], BF16, tag="arg_sb",
                                               name="arg_sb", bufs=1)
                            nc.sync.dma_start(out=arg_sb[:],
                                              in_=agA_out[t][:])
                            arT = work.tile([128, FCN * 64], FP8, tag="arT",
                                            name="arT", bufs=1)
                            for fc in range(FCN):
                                art_ps = psum.tile(
                                    [128, 64], BF16,
                                    tag=("small" if fc % 2 else "ctx"),
                                    name="art_ps", bufs=1)
                                nc.tensor.transpose(
                                    art_ps[:],
                                    arg_sb[:, fc * 128:(fc + 1) * 128],
                                    ident_s[0:B, 0:B])
                                nc.vector.tensor_copy(
                                    arT[:, fc * 64:(fc + 1) * 64],
                                    art_ps[:])

                            # ctx_ps holds 512*ctx (32 from w, 16 from a2c)
                            ctx_ps = psum.tile([B, 256], FP32, tag="ctx",
                                               name="ctx_ps", bufs=1)
                            for fc2 in range(8):
                                lhs = AP(arT[:].tensor,
                                         arT[:].offset + fc2 * 128,
                                         [list(arT[:].ap[0]),
                                          [64, 2], [1, 64]])
                                rhs = AP(a2cT_s[fc2][:].tensor,
                                         a2cT_s[fc2][:].offset,
                                         [list(a2cT_s[fc2][:].ap[0]),
                                          [256, 2], [1, 256]])
                                nc.tensor.matmul(
                                    ctx_ps[:], lhs, rhs, start=(fc2 == 0),
                                    stop=False, perf_mode=PM.DoubleRow)
                            nc.tensor.matmul(ctx_ps[:], ones64[:],
                                             a2c_b_s[:], start=False,
                                             stop=True)

                            sig3 = work.tile([B, 384], FP32, tag="sig3",
                                             name="sig3", bufs=1)
                            nc.scalar.activation(sig3[:], sums_ps[:, 0:384],
                                                 AF.Tanh, scale=0.5)
                            sitr = work.tile([B, 256], FP32, tag="sitr",
                                             name="sitr", bufs=1)
                            nc.scalar.copy(sitr[:], sums_ps[:, 384:640])
                            itr1 = work.tile([B, GC], FP32, tag="itr1",
                                             name="itr1", bufs=1)
                            nc.vector.scalar_tensor_tensor(
                                itr1[:], ctx_ps[:, 0:128], 1.0 / 512.0,
                                sitr[:, 0:128], op0=ALU.mult, op1=ALU.add)
                            itr2 = work.tile([B, GC], FP32, tag="itr2",
                                             name="itr2", bufs=1)
                            nc.vector.scalar_tensor_tensor(
                                itr2[:], ctx_ps[:, 128:256], 1.0 / 512.0,
                                sitr[:, 128:256], op0=ALU.mult, op1=ALU.add)
                            g_t = work.tile([B, GC], FP32, tag="g_t",
                                            name="g_t", bufs=1)
                            nc.vector.tensor_tensor(g_t[:], itr1[:],
                                                    itr2[:], op=ALU.max)
                            a_t = work.tile([B, GC], FP32, tag="a_t",
                                            name="a_t", bufs=1)
                            nc.vector.scalar_tensor_tensor(
                                a_t[:], sig3[:, 128:256], 1.0, c_st[:],
                                op0=ALU.add, op1=ALU.mult)
                            b_t = work.tile([B, GC], FP32, tag="b_t",
                                            name="b_t", bufs=1)
                            nc.vector.scalar_tensor_tensor(
                                b_t[:], sig3[:, 0:128], 1.0, g_t[:],
                                op0=ALU.add, op1=ALU.mult)
                            nc2_t = work.tile([B, GC], FP32, tag="nc2",
                                              name="nc2", bufs=1)
                            nc.vector.tensor_tensor(nc2_t[:], a_t[:],
                                                    b_t[:], op=ALU.add)
                            nc.vector.tensor_scalar(c_st[:], nc2_t[:], 0.5,
                                                    None, op0=ALU.mult)
                            tnc = work.tile([B, GC], FP32, tag="tnc",
                                            name="tnc", bufs=1)
                            nc.scalar.activation(tnc[:], nc2_t[:], AF.Tanh,
                                                 scale=0.5)
                            nh2 = work.tile([B, GC], BF16, tag="nh2",
                                            name="nh2", bufs=1)
                            nc.vector.scalar_tensor_tensor(
                                nh2[:], sig3[:, 256:384], 1.0, tnc[:],
                                op0=ALU.add, op1=ALU.mult)

                            nhT_ps = psum.tile([GC, B], BF16, tag="small",
                                               name="nhT_ps", bufs=1)
                            nc.tensor.transpose(nhT_ps[:], nh2[:],
                                                ident_s[0:B, 0:B])
                            nhT_sb = work.tile([GC, B], BF16, tag="nhT_sb",
                                               name="nhT_sb", bufs=1)
                            nc.vector.tensor_copy(nhT_sb[:], nhT_ps[:])
                            agH_in = dpool.tile([GC, B], BF16, tag="agH_in",
                                                name="agH_in")
                            nc.sync.dma_start(out=agH_in[:], in_=nhT_sb[:])
                            if no_cc:
                                nc.sync.dma_start(out=agH_out[t][0:GC, :],
                                                  in_=agH_in[:])
                            else:
                                nc.gpsimd.collective_compute(
                                    "AllGather", ALU.bypass,
                                    replica_groups=RG,
                                    ins=[agH_in.opt()], outs=[agH_out[t][:]])
                            # h_{t+1} -> block t//2, half t%2
                            if t % 2 == 0:
                                h2b[t // 2] = hpool.tile(
                                    [128, RCN * 128], BF16, tag="h2",
                                    name=f"h2_{t // 2}")
                                h2f8b[t // 2] = hpool.tile(
                                    [128, RCN * 128], FP8, tag="h2f8",
                                    name=f"h2f8_{t // 2}")
                            blk = h2b[t // 2]
                            dst = AP(blk[:].tensor,
                                     blk[:].offset + (t % 2) * 64,
                                     [list(blk[:].ap[0]), [128, RCN],
                                      [1, 64]])
                            nc.sync.dma_start(
                                out=dst,
                                in_=agH_out[t][:].rearrange(
                                    "(rc rl) b -> rl rc b", rc=RCN))
                            fblk = h2f8b[t // 2]
                            fsrc = AP(blk[:].tensor,
                                      blk[:].offset + (t % 2) * 64,
                                      [list(blk[:].ap[0]), [128, RCN],
                                       [1, 64]])
                            fdst = AP(fblk[:].tensor,
                                      fblk[:].offset + (t % 2) * 64,
                                      [list(fblk[:].ap[0]), [128, RCN],
                                       [1, 64]])
                            nc.vector.tensor_copy(fdst, fsrc)

                            if t == 0:
                                probe("ah0", ah_sb[:], [B, H], BF16)
                                probe("ahT0", ahT[:], [128, HCN * 8], BF16)
                                probe("u0", u[:], [BMY, L], FP32)
                                probe("statall0", stat_all[:],
                                      [128, LP], FP8)
                                probe("ar0", ar_sb[:], [BMY, F], BF16)
                                probe("arT0", arT[:], [128, FCN * 64], BF16)
                                probe("nh20", nh2[:], [B, GC], BF16)
                                probe("agH0", agH_out[0][:], [R, B], BF16)

                        # remaining pairs + stats
                        emit_logit_pair(NPAIR - 1)
                        emit_pair_stats(NPAIR - 1, work)
                        probe("lgb0", lgb[0][:], [128, VP], BF16)

                    # ---------- phase 2 ----------
                    with tc.tile_pool(name=f"p2_{rep}", bufs=2) as p2:
                        agS_in = dpool.tile([128, 2 * NT], FP32,
                                            tag="agS_in", name="agS_in")
                        nc.sync.dma_start(out=agS_in[:, 0:NT],
                                          in_=negm_all[:])
                        nc.sync.dma_start(out=agS_in[:, NT:2 * NT],
                                          in_=s_all[:])
                        if no_cc:
                            nc.sync.dma_start(out=agS_out[0:128, :],
                                              in_=agS_in[:])
                        else:
                            nc.gpsimd.collective_compute(
                                "AllGather", ALU.bypass, replica_groups=RG,
                                ins=[agS_in.opt()], outs=[agS_out[:]])
                        statg = p2.tile([128, NC * 2 * NT], FP32,
                                        tag="statg", name="statg", bufs=1)
                        nc.sync.dma_start(
                            out=statg[:].rearrange("p (r s) -> p r s", r=NC),
                            in_=agS_out[:].rearrange("(r p) s -> p r s",
                                                     r=NC))
                        sview = statg[:].rearrange("p (r s) -> p s r", r=NC)
                        negM = p2.tile([128, NT], FP32, tag="negM",
                                       name="negM", bufs=1)
                        nc.vector.tensor_reduce(negM[:], sview[:, 0:NT, :],
                                                axis=AX.X, op=ALU.min)
                        earg = p2.tile([128, NT * NC], FP32, tag="earg",
                                       name="earg", bufs=1)
                        nc.vector.tensor_tensor(
                            earg[:].rearrange("p (s r) -> p s r", r=NC),
                            bcast_free(negM[:], NC), sview[:, 0:NT, :],
                            op=ALU.subtract)
                        em = p2.tile([128, NT * NC], FP32, tag="em",
                                     name="em", bufs=1)
                        nc.scalar.activation(em[:], earg[:], AF.Exp)
                        sexp = p2.tile([128, NT * NC], FP32, tag="sexp",
                                       name="sexp", bufs=1)
                        nc.vector.tensor_tensor(
                            sexp[:].rearrange("p (s r) -> p s r", r=NC),
                            em[:].rearrange("p (s r) -> p s r", r=NC),
                            sview[:, NT:2 * NT, :], op=ALU.mult)
                        S_t = p2.tile([128, NT], FP32, tag="S_t",
                                      name="S_t", bufs=1)
                        nc.vector.tensor_reduce(
                            S_t[:],
                            sexp[:].rearrange("p (s r) -> p s r", r=NC),
                            axis=AX.X, op=ALU.add)
                        lnS = p2.tile([128, NT], FP32, tag="lnS",
                                      name="lnS", bufs=1)
                        nc.scalar.activation(lnS[:], S_t[:], AF.Ln)
                        logZ = p2.tile([128, NT], FP32, tag="logZ",
                                       name="logZ", bufs=1)
                        nc.vector.scalar_tensor_tensor(
                            logZ[:], negM[:], -1.0, lnS[:], op0=ALU.mult,
                            op1=ALU.add)
                        probe("logZ", logZ[:], [128, NT], FP32)
                        for tt in range(NT):
                            lp_t = p2.tile([128, VP], FP32, tag="p2lp",
                                           name="p2lp")
                            nc.vector.tensor_scalar(
                                lp_t[:], lgb[tt][:], logZ[:, tt:tt + 1],
                                None, op0=ALU.subtract)
                            nc.sync.dma_start(
                                out=out_d[tt * 128:(tt + 1) * 128, :],
                                in_=lp_t[:])

                for rep in range(reps):
                    emit_rep(rep)

    nc.compile()
    return nc, sorted(probes)


_NC_CACHE = {}


def kernel(**inputs):
    """Full-input entry point: returns logp [B, T, V1] float32."""
    from concourse.bass_utils import run_bass_kernel_spmd
    in_maps = host_prep(inputs)
    if "nc" not in _NC_CACHE:
        _NC_CACHE["nc"], _ = build(T, (), reps=1)
    nc = _NC_CACHE["nc"]
    res = run_bass_kernel_spmd(nc, in_maps, list(range(NC)))
    outs = [res.results[c]["logp"] for c in range(NC)]
    full = np.concatenate(outs, axis=1)[:, :V1]          # [T*B, V1]
    logp = full.reshape(T, B, V1).transpose(1, 0, 2)
    return np.ascontiguousarray(logp.astype(np.float32))


# revision 36
# speedup vs baseline: 1.0570x; 1.0148x over previous
"""Attention-LSTM captioning model on 8 trn2 cores (8-way tensor parallel).

Gate/itr/vocab output dims sharded across cores (full B=64 per core);
attention batch-sharded (8 batches/core, selected via per-core one-hot bsel
data, since the SPMD program is identical on every core). Activations are
transposed [feature, batch]. Per step: AllGather(att_res), AllGather(nh
chunk). Sigmoid(x) is computed as (tanh(x/2)+1)/2 so phase 1 only needs the
{tanh, exp} ACT table; the hidden state is stored as 2*h with h-consuming
weights pre-halved on the host.

v2: att_res matmul runs in fp8e4 DoubleRow mode (w scaled x32, a2c
pre-divided); the logit matmul is computed per step-pair with M=128 (both
steps' h in one stationary tile), scheduled one pair late so it lands in
the att_res AllGather window; logits stay in SBUF as bf16 and log_softmax
stats are folded into phase 1, so phase 2 is just one stats AllGather and
the final normalize (no DRAM scratch roundtrip).
"""
import numpy as np
import ml_dtypes

import concourse.bacc as bacc
import concourse.mybir as mybir
import concourse.tile as tile
from concourse.ap import AP
from concourse.bass_utils import run_bass_kernel_spmd

BF16_NP = ml_dtypes.bfloat16
FP8_NP = ml_dtypes.float8_e4m3
FP32 = mybir.dt.float32
BF16 = mybir.dt.bfloat16
FP8 = mybir.dt.float8e4
AF = mybir.ActivationFunctionType
ALU = mybir.AluOpType
AX = mybir.AxisListType
PM = mybir.MatmulPerfMode

B, T, R, H, F, E, L, V1 = 64, 20, 1024, 512, 2048, 300, 196, 12001
NC = 8
BMY = B // NC
GC = R // NC              # 128
NGATE = 5 * GC            # 640
VP = 1504
NG = 14                   # l-groups of 16 (224 >= L), even for fp8 pairs
LP = NG * 16              # 224
W_SCALE = 32.0
EP = 384
HCN = H // 128            # 4
FCN = F // 128            # 16
RCN = R // 128            # 8
NBL = BMY * L             # 1568


def _bf(x):
    return np.ascontiguousarray(np.asarray(x, dtype=np.float32)).astype(BF16_NP)


def _f8(x):
    return np.ascontiguousarray(np.asarray(x, dtype=np.float32)).astype(FP8_NP)


def bcast_free(ap, n):
    """Append a step-0 free dim of size n to an AP (broadcast)."""
    return AP(ap.tensor, ap.offset, list(ap.ap) + [[0, n]])


def host_prep(inputs):
    seq = np.asarray(inputs["seq"])
    att = np.asarray(inputs["att_feats"], dtype=np.float32)
    embed_w = np.asarray(inputs["embed_w"], dtype=np.float32)
    ctx2att_w = np.asarray(inputs["ctx2att_w"], dtype=np.float32)
    ctx2att_b = np.asarray(inputs["ctx2att_b"], dtype=np.float32)
    h2att_w = np.asarray(inputs["h2att_w"], dtype=np.float32)
    h2att_b = np.asarray(inputs["h2att_b"], dtype=np.float32)
    alpha_w = np.asarray(inputs["alpha_w"], dtype=np.float32)
    i2h_w = np.asarray(inputs["i2h_w"], dtype=np.float32)
    i2h_b = np.asarray(inputs["i2h_b"], dtype=np.float32)
    h2h_w = np.asarray(inputs["h2h_w"], dtype=np.float32)
    h2h_b = np.asarray(inputs["h2h_b"], dtype=np.float32)
    a2c_w = np.asarray(inputs["a2c_w"], dtype=np.float32)
    a2c_b = np.asarray(inputs["a2c_b"], dtype=np.float32)
    logit_w = np.asarray(inputs["logit_w"], dtype=np.float32)
    logit_b = np.asarray(inputs["logit_b"], dtype=np.float32)

    xt = embed_w[seq]                                    # [B, T, E]
    xtT = np.zeros((EP, T * B), dtype=np.float32)
    xtT[:E] = xt.transpose(2, 1, 0).reshape(E, T * B)
    xtT[E] = 1.0
    xtT = _bf(xtT)
    bias_gate = i2h_b + h2h_b

    in_maps = []
    for c in range(NC):
        m = {"xtT": xtT}
        grows = np.concatenate([np.arange(gg * R + c * GC, gg * R + (c + 1) * GC)
                                for gg in range(5)])
        i2hT = np.zeros((EP, NGATE), dtype=np.float32)
        i2hT[:E] = i2h_w[grows, :].T
        i2hT[E] = bias_gate[grows]
        m["i2hT"] = _bf(i2hT)
        m["h2hT"] = _bf(h2h_w[grows, :].T * 0.5)
        ht8 = (h2att_w.T * 0.5).reshape(4, 2, 128, H)
        m["h2attT_f8"] = _f8(ht8.transpose(0, 2, 1, 3).reshape(512, 2 * H))
        m["h2att_bias"] = _bf(h2att_b[None, :])
        m["ctxT"] = _bf(ctx2att_w.T)
        m["ctx_bias"] = _bf(ctx2att_b[None, :])
        amy = att[c * BMY:(c + 1) * BMY]                 # [8, L, F]
        m["attT_f8"] = _f8(amy.transpose(2, 0, 1).reshape(F, NBL))
        # ctx2att in fp8 k-tile-pair layout: [pair*128+p, i*H+h] =
        # ctx2att_w.T[pair*256+i*128+p, h]
        cT = ctx2att_w.T.reshape(8, 2, 128, H)           # [pair, i, p, h]
        m["ctxT_f8"] = _f8(cT.transpose(0, 2, 1, 3).reshape(1024, 2 * H))
        # fp8 att for the att_res matmul: part p=(b*16+lg), col=(g*F+f),
        # value att[b, g*16+lg, f]
        apad = np.zeros((BMY, LP, F), dtype=np.float32)
        apad[:, :L] = amy
        m["att_f8"] = _f8(apad.reshape(BMY, NG, 16, F).transpose(0, 2, 1, 3)
                          .reshape(128, NG * F))
        # alpha one-hot diag, fp8 k-pair layout per hc-pair:
        # [hcp*128+p, b*32+i*16+m] = 32*alpha[(2hcp+i)*128+p] iff m==b
        ac = np.zeros((2, 128, BMY, 2, 16), dtype=np.float32)
        for hcp in range(2):
            for i in range(2):
                for b in range(BMY):
                    ac[hcp, :, b, i, b] = \
                        W_SCALE * alpha_w[0, (2 * hcp + i) * 128:
                                          (2 * hcp + i + 1) * 128]
        m["alpha_f8"] = _f8(ac.reshape(256, BMY * 32))
        arows = np.concatenate([np.arange(c * GC, (c + 1) * GC),
                                np.arange(R + c * GC, R + (c + 1) * GC)])
        # a2c in fp8 k-pair layout, scaled x16 (itr add divides by 512)
        a2cp = (a2c_w[arows, :].T * 16.0).reshape(8, 2, 128, 256)
        m["a2cT_f8"] = _f8(a2cp.transpose(0, 2, 1, 3).reshape(1024, 512))
        m["a2c_bias"] = _bf(a2c_b[arows][None, :] * 512.0)
        vrows = np.arange(c * VP, (c + 1) * VP)
        lw = np.zeros((R, VP), dtype=np.float32)
        lb = np.full((1, VP), -1e30, dtype=np.float32)
        valid = vrows < V1
        lw[:, valid] = logit_w[vrows[valid], :].T * 0.5
        lb[0, valid] = logit_b[vrows[valid]]
        # fp8 k-pair layout, x8 for fp8 range (bias-add divides by 8)
        lwp = (lw * 8.0).reshape(4, 2, 128, VP)
        m["logitT_f8"] = _f8(lwp.transpose(0, 2, 1, 3).reshape(512, 2 * VP))
        m["logit_bias"] = lb
        m["ident"] = _bf(np.eye(128))
        bsel = np.zeros((B, BMY), dtype=np.float32)
        for j in range(BMY):
            bsel[c * BMY + j, j] = 1.0
        m["bsel"] = _bf(bsel)
        in_maps.append(m)
    return in_maps


def build(t_steps=T, probes=(), reps=1, no_cc=False):
    assert t_steps % 2 == 0
    nc = bacc.Bacc("TRN2", target_bir_lowering=False, debug=False,
                   num_devices=NC)
    probes = set(probes)
    NT = t_steps * B // 128
    NPAIR = t_steps // 2
    RG = [list(range(NC))]

    def din(name, shape, dt=BF16):
        return nc.dram_tensor(name, shape, dt, kind="ExternalInput")

    xtT_d = din("xtT", [EP, T * B])
    i2hT_d = din("i2hT", [EP, NGATE])
    h2hT_d = din("h2hT", [R, NGATE])
    h2attT_d = din("h2attT_f8", [512, 2 * H], FP8)
    h2att_b_d = din("h2att_bias", [1, H])
    ctxT_d = din("ctxT_f8", [1024, 2 * H], FP8)
    ctx_b_d = din("ctx_bias", [1, H])
    attT_d = din("attT_f8", [F, NBL], FP8)
    att_f8_d = din("att_f8", [128, NG * F], FP8)
    alpha_d = din("alpha_f8", [256, BMY * 32], FP8)
    a2cT_d = din("a2cT_f8", [1024, 512], FP8)
    a2c_b_d = din("a2c_bias", [1, 256])
    logitT_d = din("logitT_f8", [512, 2 * VP], FP8)
    logit_b_d = din("logit_bias", [1, VP], FP32)
    ident_d = din("ident", [128, 128])
    bsel_d = din("bsel", [B, BMY])

    out_d = nc.dram_tensor("logp", [t_steps * B, VP], FP32,
                           kind="ExternalOutput")
    agA_out_r = [[nc.dram_tensor(f"agA_out_{rp}_{t}", [B, F], BF16,
                                 addr_space="Shared") for t in range(t_steps)]
                 for rp in range(reps)]
    agH_out_r = [[nc.dram_tensor(f"agH_out_{rp}_{t}", [R, B], BF16,
                                 addr_space="Shared") for t in range(t_steps)]
                 for rp in range(reps)]
    agS_out_r = [nc.dram_tensor(f"agS_out_{rp}", [NC * 128, 2 * NT], FP32,
                                addr_space="Shared") for rp in range(reps)]

    with tile.TileContext(nc) as tc:
        with (
            tc.tile_pool(name="wpool", bufs=1) as wpool,
            tc.tile_pool(name="hpool", bufs=3) as hpool,
            tc.tile_pool(name="psum", bufs=1, space="PSUM") as psum,
            tc.tile_pool(name="dram", bufs=4, space="DRAM") as dpool,
        ):
            def probe_(name, src_ap, shape, dt):
                pd = nc.dram_tensor(f"probe_{name}", list(shape), dt,
                                    kind="ExternalOutput")
                nc.sync.dma_start(out=pd[:], in_=src_ap)

            def load_chunks(pool, dram, cols, n, tag, dt=BF16):
                ts = []
                for i in range(n):
                    t_ = pool.tile([128, cols], dt, tag=f"{tag}{i}",
                                   name=f"{tag}{i}")
                    nc.sync.dma_start(out=t_[:],
                                      in_=dram[i * 128:(i + 1) * 128, :])
                    ts.append(t_)
                return ts

            logitT_s = load_chunks(wpool, logitT_d, 2 * VP, 4, "logitT", FP8)
            logit_b_s = wpool.tile([128, VP], FP32, tag="logitb",
                                   name="logitb")
            _lb_src = AP(logit_b_d[:].tensor, logit_b_d[:].offset,
                         [[0, 128], [1, VP]])
            nc.sync.dma_start(out=logit_b_s[:], in_=_lb_src)
            ident_s = wpool.tile([128, 128], BF16, tag="ident", name="ident")
            nc.sync.dma_start(out=ident_s[:], in_=ident_d[:])
            ones64 = wpool.tile([1, B], BF16, tag="ones64", name="ones64")
            nc.vector.memset(ones64[:], 1.0)
            negm_all = wpool.tile([128, NT], FP32, tag="negm_all",
                                  name="negm_all")
            s_all = wpool.tile([128, NT], FP32, tag="s_all", name="s_all")
            # bf16 logits resident in SBUF, one tile per step-pair
            lgb = [wpool.tile([128, VP], BF16, tag=f"lgb{k}", name=f"lgb{k}")
                   for k in range(NPAIR)]

            with tc.tile_pool(name="w1pool", bufs=1) as w1pool:
                xtT_s = load_chunks(w1pool, xtT_d, T * B, 3, "xtT")
                i2hT_s = load_chunks(w1pool, i2hT_d, NGATE, 3, "i2hT")
                h2hT_s = load_chunks(w1pool, h2hT_d, NGATE, RCN, "h2hT")
                h2attT_s = load_chunks(w1pool, h2attT_d, 2 * H, 4,
                                       "h2attT", FP8)
                att_f8_s = w1pool.tile([128, NG * F], FP8, tag="attf8",
                                       name="attf8")
                nc.sync.dma_start(out=att_f8_s[:], in_=att_f8_d[:])
                alpha_s = load_chunks(w1pool, alpha_d, BMY * 32, 2,
                                      "alpha", FP8)
                a2cT_s = load_chunks(w1pool, a2cT_d, 512, 8, "a2cT", FP8)
                bsel_s = w1pool.tile([B, BMY], BF16, tag="bsel", name="bsel")
                nc.sync.dma_start(out=bsel_s[:], in_=bsel_d[:])
                h2att_b_s = w1pool.tile([1, H], BF16, tag="h2attb",
                                        name="h2attb")
                nc.sync.dma_start(out=h2att_b_s[:], in_=h2att_b_d[:])
                ctx_b_s = w1pool.tile([1, H], BF16, tag="ctxb", name="ctxb")
                nc.sync.dma_start(out=ctx_b_s[:], in_=ctx_b_d[:])
                a2c_b_s = w1pool.tile([1, 256], BF16, tag="a2cb", name="a2cb")
                nc.sync.dma_start(out=a2c_b_s[:], in_=a2c_b_d[:])
                onesNBL = w1pool.tile([1, NBL], BF16, tag="onesNBL",
                                      name="onesNBL")
                nc.vector.memset(onesNBL[:], 1.0)
                p_attT = [w1pool.tile([128, NBL], BF16, tag=f"pattT{hc}",
                                      name=f"pattT{hc}")
                          for hc in range(HCN)]
                stat_all = w1pool.tile([128, LP], FP8, tag="stat_all",
                                       name="stat_all")
                nc.vector.memset(stat_all[:], 0.0)
                w_f8 = w1pool.tile([BMY, LP], FP8, tag="w_f8", name="w_f8")
                nc.vector.memset(w_f8[:], 0.0)
                zh = w1pool.tile([128, 64], BF16, tag="zh", name="zh")
                nc.vector.memset(zh[:], 0.0)
                zf8 = w1pool.tile([128, 384], FP8, tag="zf8", name="zf8")
                nc.vector.memset(zf8[:], 0.0)
                c_st = w1pool.tile([B, GC], FP32, tag="c_st", name="c_st")

                def emit_rep(rep):
                    agA_out = agA_out_r[rep]
                    agH_out = agH_out_r[rep]
                    agS_out = agS_out_r[rep]

                    def probe(name, src_ap, shape, dt):
                        if rep == 0 and name in probes:
                            probe_(name, src_ap, shape, dt)

                    nc.vector.memset(c_st[:], 0.0)
                    # h2 blocks: [128, rc(8) x half(2) x b(64)]; block k
                    # holds h_{2k+1} (half 0) and h_{2k+2} (half 1), as 2*h.
                    # h2f8 blocks mirror them in fp8 for ah/logit matmuls.
                    h2b = [None] * NPAIR
                    h2f8b = [None] * NPAIR

                    def h_ap(j, rc):
                        """lhsT slice [128, 64] for h_j, R-chunk rc."""
                        if j == 0:
                            return zh[:]
                        blk = h2b[(j - 1) // 2]
                        c0 = rc * 128 + ((j - 1) % 2) * 64
                        return blk[:, c0:c0 + 64]

                    # ---------- phase 0 ----------
                    with (
                        tc.tile_pool(name=f"ctxpool{rep}", bufs=1) as ctxpool,
                        tc.tile_pool(name=f"stream{rep}", bufs=3) as stream,
                    ):
                        ctxT_s = load_chunks(ctxpool, ctxT_d, 2 * H, 8,
                                             "ctxT", FP8)
                        QW = 392
                        for q in range(4):
                            n0 = q * QW
                            _pa_tags = ["sums", "ah", "ar", "lg"]
                            pa_ps = [psum.tile([128, QW], FP32,
                                               tag=_pa_tags[hc],
                                               name=f"pa{hc}",
                                               bufs=(2 if hc == 3 else 1))
                                     for hc in range(HCN)]
                            for fc2 in range(8):
                                at = stream.tile([128, 2 * QW], FP8,
                                                 tag="attTq", name="attTq")
                                nc.sync.dma_start(
                                    out=at[:].rearrange(
                                        "p (two n) -> p two n", two=2),
                                    in_=attT_d[fc2 * 256:(fc2 + 1) * 256,
                                               n0:n0 + QW].rearrange(
                                        "(two p) n -> p two n", two=2))
                                for hc in range(HCN):
                                    lhs = AP(ctxT_s[fc2][:].tensor,
                                             ctxT_s[fc2][:].offset
                                             + hc * 128,
                                             [list(ctxT_s[fc2][:].ap[0]),
                                              [H, 2], [1, 128]])
                                    nc.tensor.matmul(
                                        pa_ps[hc][:], lhs,
                                        at[:].rearrange(
                                            "p (two n) -> p two n", two=2),
                                        start=(fc2 == 0), stop=False,
                                        perf_mode=PM.DoubleRow)
                            for hc in range(HCN):
                                nc.tensor.matmul(
                                    pa_ps[hc][:],
                                    ctx_b_s[:, hc * 128:(hc + 1) * 128],
                                    onesNBL[:, n0:n0 + QW], start=False,
                                    stop=True)
                                nc.vector.tensor_copy(
                                    p_attT[hc][:, n0:n0 + QW], pa_ps[hc][:])
                    probe("p_attT0", p_attT[0][:], [128, NBL], BF16)

                    def emit_logit_pair(k):
                        """Logit matmul for step pair k from h2b[k] (M=128),
                        bias-add into lgb[k] (bf16)."""
                        fblk = h2f8b[k]
                        for ci, c0 in enumerate((0, 512, 1024)):
                            c1 = min(VP, c0 + 512)
                            lg_ps = psum.tile([128, 512], FP32, tag="lg",
                                              name="lg_ps", bufs=2)
                            for rc2 in range(4):
                                lhs = AP(fblk[:].tensor,
                                         fblk[:].offset + rc2 * 256,
                                         [list(fblk[:].ap[0]),
                                          [128, 2], [1, 128]])
                                rhs = AP(logitT_s[rc2][:].tensor,
                                         logitT_s[rc2][:].offset + c0,
                                         [list(logitT_s[rc2][:].ap[0]),
                                          [VP, 2], [1, c1 - c0]])
                                nc.tensor.matmul(
                                    lg_ps[:, 0:c1 - c0], lhs, rhs,
                                    start=(rc2 == 0), stop=(rc2 == 3),
                                    perf_mode=PM.DoubleRow)
                            nc.vector.scalar_tensor_tensor(
                                lgb[k][:, c0:c1], lg_ps[:, 0:c1 - c0],
                                1.0 / 8.0, logit_b_s[:, c0:c1],
                                op0=ALU.mult, op1=ALU.add)

                    def emit_pair_stats(k, work):
                        """Softmax stats for pair k (reads lgb[k], SBUF)."""
                        nc.vector.tensor_reduce(
                            negm_all[:, k:k + 1], lgb[k][:], axis=AX.X,
                            op=ALU.max, negate=True)
                        junk = work.tile([128, VP], BF16, tag="p2junk",
                                         name="p2junk", bufs=2)
                        nc.scalar.activation(
                            junk[:], lgb[k][:], AF.Exp,
                            bias=negm_all[:, k:k + 1],
                            accum_out=s_all[:, k:k + 1])

                    # ---------- phase 1 ----------
                    with tc.tile_pool(name=f"work1_{rep}", bufs=1) as work:
                        for t in range(t_steps):
                            # stats for the pair computed two steps ago run
                            # in this step's sums/ah window (DVE+ACT idle)
                            if t >= 3 and t % 2 == 1:
                                emit_pair_stats((t - 3) // 2, work)
                            sums_ps = psum.tile([B, NGATE], FP32, tag="sums",
                                                name="sums", bufs=1)
                            for c0 in (0, 512):
                                c1 = min(NGATE, c0 + 512)
                                for kc in range(3):
                                    nc.tensor.matmul(
                                        sums_ps[:, c0:c1],
                                        xtT_s[kc][:, t * B:(t + 1) * B],
                                        i2hT_s[kc][:, c0:c1],
                                        start=(kc == 0), stop=False)
                                for rc in range(RCN):
                                    nc.tensor.matmul(
                                        sums_ps[:, c0:c1],
                                        h_ap(t, rc),
                                        h2hT_s[rc][:, c0:c1],
                                        start=False, stop=(rc == RCN - 1))

                            ah_ps = psum.tile([B, H], FP32, tag="ah",
                                              name="ah", bufs=1)
                            for rc2 in range(4):
                                if t == 0:
                                    lhs = AP(zf8[:].tensor, zf8[:].offset,
                                             [list(zf8[:].ap[0]),
                                              [128, 2], [1, 64]])
                                else:
                                    fb = h2f8b[(t - 1) // 2]
                                    lhs = AP(fb[:].tensor,
                                             fb[:].offset + rc2 * 256
                                             + ((t - 1) % 2) * 64,
                                             [list(fb[:].ap[0]),
                                              [128, 2], [1, 64]])
                                rhs = AP(h2attT_s[rc2][:].tensor,
                                         h2attT_s[rc2][:].offset,
                                         [list(h2attT_s[rc2][:].ap[0]),
                                          [H, 2], [1, H]])
                                nc.tensor.matmul(ah_ps[:], lhs, rhs,
                                                 start=(rc2 == 0),
                                                 stop=False,
                                                 perf_mode=PM.DoubleRow)
                            nc.tensor.matmul(ah_ps[:], ones64[:],
                                             h2att_b_s[:], start=False,
                                             stop=True)
                            ah_sb = work.tile([B, H], BF16, tag="ah_sb",
                                              name="ah_sb", bufs=1)
                            nc.scalar.copy(ah_sb[:], ah_ps[:])
                            ahT_ps = psum.tile([128, HCN * 8], FP32,
                                               tag="small", name="ahT_ps",
                                               bufs=1)
                            for hc in range(HCN):
                                nc.tensor.matmul(
                                    ahT_ps[:, hc * 8:(hc + 1) * 8],
                                    ah_sb[:, hc * 128:(hc + 1) * 128],
                                    bsel_s[:], start=True, stop=True)
                            ahT = work.tile([128, HCN * 8], BF16,
                                            tag="ahT_sb", name="ahT_sb",
                                            bufs=1)
                            nc.vector.tensor_copy(ahT[:], ahT_ps[:])

                            e_ps = psum.tile([BMY, L], FP32, tag="small",
                                             name="e_ps", bufs=1)
                            for hcp in range(2):
                                dt2 = work.tile([128, 2 * NBL], FP8,
                                                tag="dt2", name="dt2",
                                                bufs=2)
                                for i in range(2):
                                    hc = 2 * hcp + i
                                    dp = work.tile([128, NBL], BF16,
                                                   tag="dp", name="dp",
                                                   bufs=2)
                                    eng = nc.vector if i == 0 else nc.gpsimd
                                    eng.tensor_tensor(
                                        dp[:].rearrange("p (b l) -> p b l",
                                                        b=BMY),
                                        p_attT[hc][:].rearrange(
                                            "p (b l) -> p b l", b=BMY),
                                        bcast_free(
                                            ahT[:, hc * 8:(hc + 1) * 8], L),
                                        op=ALU.add)
                                    nc.scalar.activation(
                                        dt2[:, i * NBL:(i + 1) * NBL],
                                        dp[:], AF.Tanh)
                                for b in range(BMY):
                                    lhs = AP(alpha_s[hcp][:].tensor,
                                             alpha_s[hcp][:].offset + b * 32,
                                             [list(alpha_s[hcp][:].ap[0]),
                                              [16, 2], [1, BMY]])
                                    rhs = AP(dt2[:].tensor,
                                             dt2[:].offset + b * L,
                                             [list(dt2[:].ap[0]),
                                              [NBL, 2], [1, L]])
                                    nc.tensor.matmul(
                                        e_ps[:], lhs, rhs,
                                        start=(hcp == 0 and b == 0),
                                        stop=(hcp == 1 and b == BMY - 1),
                                        perf_mode=PM.DoubleRow)

                            # e_ps holds 32*e; |e| <~ 3 so exp(e) is safe
                            # without max-subtraction (alpha ~N(0,.02))
                            u = work.tile([BMY, L], FP32, tag="u", name="u",
                                          bufs=1)
                            ssum = work.tile([BMY, 1], FP32, tag="ssum",
                                             name="ssum", bufs=1)
                            nc.scalar.activation(u[:], e_ps[:], AF.Exp,
                                                 scale=1.0 / W_SCALE,
                                                 accum_out=ssum[:])
                            rinv = work.tile([BMY, 1], FP32, tag="rinv",
                                             name="rinv", bufs=1)
                            nc.vector.reciprocal(rinv[:], ssum[:])
                            nc.vector.tensor_scalar(w_f8[:, 0:L], u[:],
                                                    rinv[:], W_SCALE,
                                                    op0=ALU.mult,
                                                    op1=ALU.mult)

                            wdr = dpool.tile([BMY, LP], FP8, tag="wdr",
                                             name="wdr")
                            nc.sync.dma_start(out=wdr[:], in_=w_f8[:])
                            for b in range(BMY):
                                nc.sync.dma_start(
                                    out=stat_all[b * 16:(b + 1) * 16,
                                                 b:LP:16],
                                    in_=wdr[b:b + 1, :].rearrange(
                                        "o (g lp) -> (o lp) g", g=NG))

                            # att_res: fp8 DoubleRow, 7 k-tile pairs
                            ar_sb = work.tile([BMY, F], BF16, tag="ar_sb",
                                              name="ar_sb", bufs=1)
                            for fq in range(4):
                                f0 = fq * 512
                                ar_ps = psum.tile([BMY, 512], FP32,
                                                  tag="ar", name="ar_ps",
                                                  bufs=1)
                                for q in range(NG // 2):
                                    lhs = AP(stat_all[:].tensor,
                                             stat_all[:].offset + q * 32,
                                             [list(stat_all[:].ap[0]),
                                              [16, 2], [1, BMY]])
                                    rhs = AP(att_f8_s[:].tensor,
                                             att_f8_s[:].offset
                                             + 2 * q * F + f0,
                                             [list(att_f8_s[:].ap[0]),
                                              [F, 2], [1, 512]])
                                    nc.tensor.matmul(
                                        ar_ps[:], lhs, rhs,
                                        start=(q == 0),
                                        stop=(q == NG // 2 - 1),
                                        perf_mode=PM.DoubleRow)
                                nc.scalar.copy(
                                    ar_sb[:, f0:f0 + 512], ar_ps[:])
                            agA_in = dpool.tile([BMY, F], BF16, tag="agA_in",
                                                name="agA_in")
                            nc.sync.dma_start(out=agA_in[:], in_=ar_sb[:])
                            if no_cc:
                                nc.sync.dma_start(out=agA_out[t][0:BMY, :],
                                                  in_=agA_in[:])
                            else:
                                nc.gpsimd.collective_compute(
                                    "AllGather", ALU.bypass,
                                    replica_groups=RG,
                                    ins=[agA_in.opt()], outs=[agA_out[t][:]])

                            # pre-AG gate work runs in the AllGather window
                            sig3 = work.tile([B, 384], FP32, tag="sig3",
                                             name="sig3", bufs=1)
                            nc.scalar.activation(sig3[:], sums_ps[:, 0:384],
                                                 AF.Tanh, scale=0.5)
                            sitr = work.tile([B, 256], FP32, tag="sitr",
                                             name="sitr", bufs=1)
                            nc.scalar.copy(sitr[:], sums_ps[:, 384:640])
                            a_t = work.tile([B, GC], FP32, tag="a_t",
                                            name="a_t", bufs=1)
                            nc.vector.scalar_tensor_tensor(
                                a_t[:], sig3[:, 128:256], 1.0, c_st[:],
                                op0=ALU.add, op1=ALU.mult)

                            # paired logit for block (t-2)//2 runs in the
                            # AllGather window
                            if t >= 2 and t % 2 == 0:
                                emit_logit_pair((t - 2) // 2)

                            arg_sb = work.tile([B, F], BF16, tag="arg_sb",
                                               name="arg_sb", bufs=1)
                            nc.sync.dma_start(out=arg_sb[:],
                                              in_=agA_out[t][:])
                            arT = work.tile([128, FCN * 64], FP8, tag="arT",
                                            name="arT", bufs=1)
                            for fc in range(FCN):
                                art_ps = psum.tile(
                                    [128, 64], BF16,
                                    tag=("small" if fc % 2 else "ctx"),
                                    name="art_ps", bufs=1)
                                nc.tensor.transpose(
                                    art_ps[:],
                                    arg_sb[:, fc * 128:(fc + 1) * 128],
                                    ident_s[0:B, 0:B])
                                nc.vector.tensor_copy(
                                    arT[:, fc * 64:(fc + 1) * 64],
                                    art_ps[:])

                            # ctx_ps holds 512*ctx (32 from w, 16 from a2c)
                            ctx_ps = psum.tile([B, 256], FP32, tag="ctx",
                                               name="ctx_ps", bufs=1)
                            for fc2 in range(8):
                                lhs = AP(arT[:].tensor,
                                         arT[:].offset + fc2 * 128,
                                         [list(arT[:].ap[0]),
                                          [64, 2], [1, 64]])
                                rhs = AP(a2cT_s[fc2][:].tensor,
                                         a2cT_s[fc2][:].offset,
                                         [list(a2cT_s[fc2][:].ap[0]),
                                          [256, 2], [1, 256]])
                                nc.tensor.matmul(
                                    ctx_ps[:], lhs, rhs, start=(fc2 == 0),
                                    stop=False, perf_mode=PM.DoubleRow)
                            nc.tensor.matmul(ctx_ps[:], ones64[:],
                                             a2c_b_s[:], start=False,
                                             stop=True)

                            itr1 = work.tile([B, GC], FP32, tag="itr1",
                                             name="itr1", bufs=1)
                            nc.vector.scalar_tensor_tensor(
                                itr1[:], ctx_ps[:, 0:128], 1.0 / 512.0,
                                sitr[:, 0:128], op0=ALU.mult, op1=ALU.add)
                            itr2 = work.tile([B, GC], FP32, tag="itr2",
                                             name="itr2", bufs=1)
                            nc.vector.scalar_tensor_tensor(
                                itr2[:], ctx_ps[:, 128:256], 1.0 / 512.0,
                                sitr[:, 128:256], op0=ALU.mult, op1=ALU.add)
                            g_t = work.tile([B, GC], FP32, tag="g_t",
                                            name="g_t", bufs=1)
                            nc.vector.tensor_tensor(g_t[:], itr1[:],
                                                    itr2[:], op=ALU.max)
                            b_t = work.tile([B, GC], FP32, tag="b_t",
                                            name="b_t", bufs=1)
                            nc.vector.scalar_tensor_tensor(
                                b_t[:], sig3[:, 0:128], 1.0, g_t[:],
                                op0=ALU.add, op1=ALU.mult)
                            nc2_t = work.tile([B, GC], FP32, tag="nc2",
                                              name="nc2", bufs=1)
                            nc.vector.tensor_tensor(nc2_t[:], a_t[:],
                                                    b_t[:], op=ALU.add)
                            nc.vector.tensor_scalar(c_st[:], nc2_t[:], 0.5,
                                                    None, op0=ALU.mult)
                            tnc = work.tile([B, GC], FP32, tag="tnc",
                                            name="tnc", bufs=1)
                            nc.scalar.activation(tnc[:], nc2_t[:], AF.Tanh,
                                                 scale=0.5)
                            nh2 = work.tile([B, GC], BF16, tag="nh2",
                                            name="nh2", bufs=1)
                            nc.vector.scalar_tensor_tensor(
                                nh2[:], sig3[:, 256:384], 1.0, tnc[:],
                                op0=ALU.add, op1=ALU.mult)

                            nhT_ps = psum.tile([GC, B], BF16, tag="small",
                                               name="nhT_ps", bufs=1)
                            nc.tensor.transpose(nhT_ps[:], nh2[:],
                                                ident_s[0:B, 0:B])
                            nhT_sb = work.tile([GC, B], BF16, tag="nhT_sb",
                                               name="nhT_sb", bufs=1)
                            nc.vector.tensor_copy(nhT_sb[:], nhT_ps[:])
                            agH_in = dpool.tile([GC, B], BF16, tag="agH_in",
                                                name="agH_in")
                            nc.sync.dma_start(out=agH_in[:], in_=nhT_sb[:])
                            if no_cc:
                                nc.sync.dma_start(out=agH_out[t][0:GC, :],
                                                  in_=agH_in[:])
                            else:
                                nc.gpsimd.collective_compute(
                                    "AllGather", ALU.bypass,
                                    replica_groups=RG,
                                    ins=[agH_in.opt()], outs=[agH_out[t][:]])
                            # h_{t+1} -> block t//2, half t%2
                            if t % 2 == 0:
                                h2b[t // 2] = hpool.tile(
                                    [128, RCN * 128], BF16, tag="h2",
                                    name=f"h2_{t // 2}")
                                h2f8b[t // 2] = hpool.tile(
                                    [128, RCN * 128], FP8, tag="h2f8",
                                    name=f"h2f8_{t // 2}")
                            blk = h2b[t // 2]
                            dst = AP(blk[:].tensor,
                                     blk[:].offset + (t % 2) * 64,
                                     [list(blk[:].ap[0]), [128, RCN],
                                      [1, 64]])
                            nc.sync.dma_start(
                                out=dst,
                                in_=agH_out[t][:].rearrange(
                                    "(rc rl) b -> rl rc b", rc=RCN))
                            fblk = h2f8b[t // 2]
                            fsrc = AP(blk[:].tensor,
                                      blk[:].offset + (t % 2) * 64,
                                      [list(blk[:].ap[0]), [128, RCN],
                                       [1, 64]])
                            fdst = AP(fblk[:].tensor,
                                      fblk[:].offset + (t % 2) * 64,
                                      [list(fblk[:].ap[0]), [128, RCN],
                                       [1, 64]])
                            nc.vector.tensor_copy(fdst, fsrc)

                            if t == 0:
                                probe("ah0", ah_sb[:], [B, H], BF16)
                                probe("ahT0", ahT[:], [128, HCN * 8], BF16)
                                probe("u0", u[:], [BMY, L], FP32)
                                probe("statall0", stat_all[:],
                                      [128, LP], FP8)
                                probe("ar0", ar_sb[:], [BMY, F], BF16)
                                probe("arT0", arT[:], [128, FCN * 64], BF16)
                                probe("nh20", nh2[:], [B, GC], BF16)
                                probe("agH0", agH_out[0][:], [R, B], BF16)

                        # remaining pairs + stats
                        emit_logit_pair(NPAIR - 1)
                        emit_pair_stats(NPAIR - 1, work)
                        probe("lgb0", lgb[0][:], [128, VP], BF16)

                    # ---------- phase 2 ----------
                    with tc.tile_pool(name=f"p2_{rep}", bufs=2) as p2:
                        agS_in = dpool.tile([128, 2 * NT], FP32,
                                            tag="agS_in", name="agS_in")
                        nc.sync.dma_start(out=agS_in[:, 0:NT],
                                          in_=negm_all[:])
                        nc.sync.dma_start(out=agS_in[:, NT:2 * NT],
                                          in_=s_all[:])
                        if no_cc:
                            nc.sync.dma_start(out=agS_out[0:128, :],
                                              in_=agS_in[:])
                        else:
                            nc.gpsimd.collective_compute(
                                "AllGather", ALU.bypass, replica_groups=RG,
                                ins=[agS_in.opt()], outs=[agS_out[:]])
                        statg = p2.tile([128, NC * 2 * NT], FP32,
                                        tag="statg", name="statg", bufs=1)
                        nc.sync.dma_start(
                            out=statg[:].rearrange("p (r s) -> p r s", r=NC),
                            in_=agS_out[:].rearrange("(r p) s -> p r s",
                                                     r=NC))
                        sview = statg[:].rearrange("p (r s) -> p s r", r=NC)
                        negM = p2.tile([128, NT], FP32, tag="negM",
                                       name="negM", bufs=1)
                        nc.vector.tensor_reduce(negM[:], sview[:, 0:NT, :],
                                                axis=AX.X, op=ALU.min)
                        earg = p2.tile([128, NT * NC], FP32, tag="earg",
                                       name="earg", bufs=1)
                        nc.vector.tensor_tensor(
                            earg[:].rearrange("p (s r) -> p s r", r=NC),
                            bcast_free(negM[:], NC), sview[:, 0:NT, :],
                            op=ALU.subtract)
                        em = p2.tile([128, NT * NC], FP32, tag="em",
                                     name="em", bufs=1)
                        nc.scalar.activation(em[:], earg[:], AF.Exp)
                        sexp = p2.tile([128, NT * NC], FP32, tag="sexp",
                                       name="sexp", bufs=1)
                        nc.vector.tensor_tensor(
                            sexp[:].rearrange("p (s r) -> p s r", r=NC),
                            em[:].rearrange("p (s r) -> p s r", r=NC),
                            sview[:, NT:2 * NT, :], op=ALU.mult)
                        S_t = p2.tile([128, NT], FP32, tag="S_t",
                                      name="S_t", bufs=1)
                        nc.vector.tensor_reduce(
                            S_t[:],
                            sexp[:].rearrange("p (s r) -> p s r", r=NC),
                            axis=AX.X, op=ALU.add)
                        lnS = p2.tile([128, NT], FP32, tag="lnS",
                                      name="lnS", bufs=1)
                        nc.scalar.activation(lnS[:], S_t[:], AF.Ln)
                        logZ = p2.tile([128, NT], FP32, tag="logZ",
                                       name="logZ", bufs=1)
                        nc.vector.scalar_tensor_tensor(
                            logZ[:], negM[:], -1.0, lnS[:], op0=ALU.mult,
                            op1=ALU.add)
                        probe("logZ", logZ[:], [128, NT], FP32)
                        for tt in range(NT):
                            lp_t = p2.tile([128, VP], FP32, tag="p2lp",
                                           name="p2lp", bufs=4)
                            eng = nc.vector if tt % 2 == 0 else nc.gpsimd
                            eng.tensor_scalar(
                                lp_t[:], lgb[tt][:], logZ[:, tt:tt + 1],
                                None, op0=ALU.subtract)
                            nc.sync.dma_start(
                                out=out_d[tt * 128:(tt + 1) * 128, :],
                                in_=lp_t[:])

                for rep in range(reps):
                    emit_rep(rep)

    nc.compile()
    return nc, sorted(probes)


_NC_CACHE = {}


def kernel(**inputs):
    """Full-input entry point: returns logp [B, T, V1] float32."""
    from concourse.bass_utils import run_bass_kernel_spmd
    in_maps = host_prep(inputs)
    if "nc" not in _NC_CACHE:
        _NC_CACHE["nc"], _ = build(T, (), reps=1)
    nc = _NC_CACHE["nc"]
    res = run_bass_kernel_spmd(nc, in_maps, list(range(NC)))
    outs = [res.results[c]["logp"] for c in range(NC)]
    full = np.concatenate(outs, axis=1)[:, :V1]          # [T*B, V1]
    logp = full.reshape(T, B, V1).transpose(1, 0, 2)
    return np.ascontiguousarray(logp.astype(np.float32))


# revision 45
# speedup vs baseline: 1.0946x; 1.0356x over previous
"""Attention-LSTM captioning model on 8 trn2 cores (8-way tensor parallel).

Gate/itr/vocab output dims sharded across cores (full B=64 per core);
attention batch-sharded (8 batches/core, selected via per-core one-hot bsel
data, since the SPMD program is identical on every core). Activations are
transposed [feature, batch]. Per step: AllGather(att_res), AllGather(nh
chunk). Sigmoid(x) is computed as (tanh(x/2)+1)/2 so phase 1 only needs the
{tanh, exp} ACT table; the hidden state is stored as 2*h with h-consuming
weights pre-halved on the host.

v2: att_res matmul runs in fp8e4 DoubleRow mode (w scaled x32, a2c
pre-divided); the logit matmul is computed per step-pair with M=128 (both
steps' h in one stationary tile), scheduled one pair late so it lands in
the att_res AllGather window; logits stay in SBUF as bf16 and log_softmax
stats are folded into phase 1, so phase 2 is just one stats AllGather and
the final normalize (no DRAM scratch roundtrip).
"""
import numpy as np
import ml_dtypes

import concourse.bacc as bacc
import concourse.mybir as mybir
import concourse.tile as tile
from concourse.ap import AP
from concourse.bass_utils import run_bass_kernel_spmd

BF16_NP = ml_dtypes.bfloat16
FP8_NP = ml_dtypes.float8_e4m3
FP32 = mybir.dt.float32
BF16 = mybir.dt.bfloat16
FP8 = mybir.dt.float8e4
AF = mybir.ActivationFunctionType
ALU = mybir.AluOpType
AX = mybir.AxisListType
PM = mybir.MatmulPerfMode

B, T, R, H, F, E, L, V1 = 64, 20, 1024, 512, 2048, 300, 196, 12001
NC = 8
BMY = B // NC
GC = R // NC              # 128
NGATE = 5 * GC            # 640
VP = 1504
NG = 14                   # l-groups of 16 (224 >= L), even for fp8 pairs
LP = NG * 16              # 224
W_SCALE = 32.0
EP = 384
HCN = H // 128            # 4
FCN = F // 128            # 16
RCN = R // 128            # 8
NBL = BMY * L             # 1568


def _bf(x):
    return np.ascontiguousarray(np.asarray(x, dtype=np.float32)).astype(BF16_NP)


def _f8(x):
    return np.ascontiguousarray(np.asarray(x, dtype=np.float32)).astype(FP8_NP)


def bcast_free(ap, n):
    """Append a step-0 free dim of size n to an AP (broadcast)."""
    return AP(ap.tensor, ap.offset, list(ap.ap) + [[0, n]])


def host_prep(inputs):
    seq = np.asarray(inputs["seq"])
    att = np.asarray(inputs["att_feats"], dtype=np.float32)
    embed_w = np.asarray(inputs["embed_w"], dtype=np.float32)
    ctx2att_w = np.asarray(inputs["ctx2att_w"], dtype=np.float32)
    ctx2att_b = np.asarray(inputs["ctx2att_b"], dtype=np.float32)
    h2att_w = np.asarray(inputs["h2att_w"], dtype=np.float32)
    h2att_b = np.asarray(inputs["h2att_b"], dtype=np.float32)
    alpha_w = np.asarray(inputs["alpha_w"], dtype=np.float32)
    i2h_w = np.asarray(inputs["i2h_w"], dtype=np.float32)
    i2h_b = np.asarray(inputs["i2h_b"], dtype=np.float32)
    h2h_w = np.asarray(inputs["h2h_w"], dtype=np.float32)
    h2h_b = np.asarray(inputs["h2h_b"], dtype=np.float32)
    a2c_w = np.asarray(inputs["a2c_w"], dtype=np.float32)
    a2c_b = np.asarray(inputs["a2c_b"], dtype=np.float32)
    logit_w = np.asarray(inputs["logit_w"], dtype=np.float32)
    logit_b = np.asarray(inputs["logit_b"], dtype=np.float32)

    xt = embed_w[seq]                                    # [B, T, E]
    xtT = np.zeros((EP, T * B), dtype=np.float32)
    xtT[:E] = xt.transpose(2, 1, 0).reshape(E, T * B)
    xtT[E] = 1.0
    xtT = _bf(xtT)
    bias_gate = i2h_b + h2h_b

    in_maps = []
    for c in range(NC):
        m = {"xtT": xtT}
        grows = np.concatenate([np.arange(gg * R + c * GC, gg * R + (c + 1) * GC)
                                for gg in range(5)])
        i2hT = np.zeros((EP, NGATE), dtype=np.float32)
        i2hT[:E] = i2h_w[grows, :].T
        i2hT[E] = bias_gate[grows]
        m["i2hT"] = _bf(i2hT)
        m["h2hT"] = _bf(h2h_w[grows, :].T * 0.5)
        ht8 = (h2att_w.T * 0.5).reshape(4, 2, 128, H)
        m["h2attT_f8"] = _f8(ht8.transpose(0, 2, 1, 3).reshape(512, 2 * H))
        m["h2att_bias"] = _bf(h2att_b[None, :])
        m["ctxT"] = _bf(ctx2att_w.T)
        m["ctx_bias"] = _bf(ctx2att_b[None, :])
        amy = att[c * BMY:(c + 1) * BMY]                 # [8, L, F]
        m["attT_f8"] = _f8(amy.transpose(2, 0, 1).reshape(F, NBL))
        # ctx2att in fp8 k-tile-pair layout: [pair*128+p, i*H+h] =
        # ctx2att_w.T[pair*256+i*128+p, h]
        cT = ctx2att_w.T.reshape(8, 2, 128, H)           # [pair, i, p, h]
        m["ctxT_f8"] = _f8(cT.transpose(0, 2, 1, 3).reshape(1024, 2 * H))
        # fp8 att for the att_res matmul: part p=(b*16+lg), col=(g*F+f),
        # value att[b, g*16+lg, f]
        apad = np.zeros((BMY, LP, F), dtype=np.float32)
        apad[:, :L] = amy
        m["att_f8"] = _f8(apad.reshape(BMY, NG, 16, F).transpose(0, 2, 1, 3)
                          .reshape(128, NG * F))
        # alpha one-hot diag, fp8 k-pair layout per hc-pair:
        # [hcp*128+p, b*32+i*16+m] = 32*alpha[(2hcp+i)*128+p] iff m==b
        ac = np.zeros((2, 128, BMY, 2, 16), dtype=np.float32)
        for hcp in range(2):
            for i in range(2):
                for b in range(BMY):
                    ac[hcp, :, b, i, b] = \
                        W_SCALE * alpha_w[0, (2 * hcp + i) * 128:
                                          (2 * hcp + i + 1) * 128]
        m["alpha_f8"] = _f8(ac.reshape(256, BMY * 32))
        arows = np.concatenate([np.arange(c * GC, (c + 1) * GC),
                                np.arange(R + c * GC, R + (c + 1) * GC)])
        # a2c in fp8 k-pair layout, scaled x16 (itr add divides by 512)
        a2cp = (a2c_w[arows, :].T * 16.0).reshape(8, 2, 128, 256)
        m["a2cT_f8"] = _f8(a2cp.transpose(0, 2, 1, 3).reshape(1024, 512))
        m["a2c_bias"] = _bf(a2c_b[arows][None, :] * 512.0)
        vrows = np.arange(c * VP, (c + 1) * VP)
        lw = np.zeros((R, VP), dtype=np.float32)
        lb = np.full((1, VP), -1e30, dtype=np.float32)
        valid = vrows < V1
        lw[:, valid] = logit_w[vrows[valid], :].T * 0.5
        lb[0, valid] = logit_b[vrows[valid]]
        # fp8 k-pair layout, x8 for fp8 range (bias-add divides by 8)
        lwp = (lw * 8.0).reshape(4, 2, 128, VP)
        m["logitT_f8"] = _f8(lwp.transpose(0, 2, 1, 3).reshape(512, 2 * VP))
        m["logit_bias"] = lb
        m["ident"] = _bf(np.eye(128))
        bsel = np.zeros((B, BMY), dtype=np.float32)
        for j in range(BMY):
            bsel[c * BMY + j, j] = 1.0
        m["bsel"] = _bf(bsel)
        in_maps.append(m)
    return in_maps


def build(t_steps=T, probes=(), reps=1, no_cc=False):
    assert t_steps % 2 == 0
    nc = bacc.Bacc("TRN2", target_bir_lowering=False, debug=False,
                   num_devices=NC)
    probes = set(probes)
    NT = t_steps * B // 128
    NPAIR = t_steps // 2
    RG = [list(range(NC))]

    def din(name, shape, dt=BF16):
        return nc.dram_tensor(name, shape, dt, kind="ExternalInput")

    xtT_d = din("xtT", [EP, T * B])
    i2hT_d = din("i2hT", [EP, NGATE])
    h2hT_d = din("h2hT", [R, NGATE])
    h2attT_d = din("h2attT_f8", [512, 2 * H], FP8)
    h2att_b_d = din("h2att_bias", [1, H])
    ctxT_d = din("ctxT_f8", [1024, 2 * H], FP8)
    ctx_b_d = din("ctx_bias", [1, H])
    attT_d = din("attT_f8", [F, NBL], FP8)
    att_f8_d = din("att_f8", [128, NG * F], FP8)
    alpha_d = din("alpha_f8", [256, BMY * 32], FP8)
    a2cT_d = din("a2cT_f8", [1024, 512], FP8)
    a2c_b_d = din("a2c_bias", [1, 256])
    logitT_d = din("logitT_f8", [512, 2 * VP], FP8)
    logit_b_d = din("logit_bias", [1, VP], FP32)
    ident_d = din("ident", [128, 128])
    bsel_d = din("bsel", [B, BMY])

    out_d = nc.dram_tensor("logp", [t_steps * B, VP], FP32,
                           kind="ExternalOutput")
    agA_out_r = [[nc.dram_tensor(f"agA_out_{rp}_{t}", [B, F], BF16,
                                 addr_space="Shared") for t in range(t_steps)]
                 for rp in range(reps)]
    agH_out_r = [[nc.dram_tensor(f"agH_out_{rp}_{t}", [R, B], BF16,
                                 addr_space="Shared") for t in range(t_steps)]
                 for rp in range(reps)]
    agS_out_r = [nc.dram_tensor(f"agS_out_{rp}", [NC * 128, 2 * NT], FP32,
                                addr_space="Shared") for rp in range(reps)]

    with tile.TileContext(nc) as tc:
        with (
            tc.tile_pool(name="wpool", bufs=1) as wpool,
            tc.tile_pool(name="hpool", bufs=3) as hpool,
            tc.tile_pool(name="psum", bufs=1, space="PSUM") as psum,
            tc.tile_pool(name="dram", bufs=4, space="DRAM") as dpool,
        ):
            def probe_(name, src_ap, shape, dt):
                pd = nc.dram_tensor(f"probe_{name}", list(shape), dt,
                                    kind="ExternalOutput")
                nc.sync.dma_start(out=pd[:], in_=src_ap)

            def load_chunks(pool, dram, cols, n, tag, dt=BF16):
                ts = []
                for i in range(n):
                    t_ = pool.tile([128, cols], dt, tag=f"{tag}{i}",
                                   name=f"{tag}{i}")
                    nc.sync.dma_start(out=t_[:],
                                      in_=dram[i * 128:(i + 1) * 128, :])
                    ts.append(t_)
                return ts

            logitT_s = load_chunks(wpool, logitT_d, 2 * VP, 4, "logitT", FP8)
            logit_b_s = wpool.tile([128, VP], FP32, tag="logitb",
                                   name="logitb")
            _lb_src = AP(logit_b_d[:].tensor, logit_b_d[:].offset,
                         [[0, 128], [1, VP]])
            nc.sync.dma_start(out=logit_b_s[:], in_=_lb_src)
            ident_s = wpool.tile([128, 128], BF16, tag="ident", name="ident")
            nc.sync.dma_start(out=ident_s[:], in_=ident_d[:])
            ones64 = wpool.tile([1, B], BF16, tag="ones64", name="ones64")
            nc.vector.memset(ones64[:], 1.0)
            negm_all = wpool.tile([128, NT], FP32, tag="negm_all",
                                  name="negm_all")
            s_all = wpool.tile([128, NT], FP32, tag="s_all", name="s_all")
            # bf16 logits resident in SBUF, one tile per step-pair
            lgb = [wpool.tile([128, VP], BF16, tag=f"lgb{k}", name=f"lgb{k}")
                   for k in range(NPAIR)]

            with tc.tile_pool(name="w1pool", bufs=1) as w1pool:
                xtT_s = load_chunks(w1pool, xtT_d, T * B, 3, "xtT")
                i2hT_s = load_chunks(w1pool, i2hT_d, NGATE, 3, "i2hT")
                h2hT_s = load_chunks(w1pool, h2hT_d, NGATE, RCN, "h2hT")
                h2attT_s = load_chunks(w1pool, h2attT_d, 2 * H, 4,
                                       "h2attT", FP8)
                att_f8_s = w1pool.tile([128, NG * F], FP8, tag="attf8",
                                       name="attf8")
                nc.sync.dma_start(out=att_f8_s[:], in_=att_f8_d[:])
                alpha_s = load_chunks(w1pool, alpha_d, BMY * 32, 2,
                                      "alpha", FP8)
                a2cT_s = load_chunks(w1pool, a2cT_d, 512, 8, "a2cT", FP8)
                bsel_s = w1pool.tile([B, BMY], BF16, tag="bsel", name="bsel")
                nc.sync.dma_start(out=bsel_s[:], in_=bsel_d[:])
                h2att_b_s = w1pool.tile([1, H], BF16, tag="h2attb",
                                        name="h2attb")
                nc.sync.dma_start(out=h2att_b_s[:], in_=h2att_b_d[:])
                ctx_b_s = w1pool.tile([1, H], BF16, tag="ctxb", name="ctxb")
                nc.sync.dma_start(out=ctx_b_s[:], in_=ctx_b_d[:])
                a2c_b_s = w1pool.tile([1, 256], BF16, tag="a2cb", name="a2cb")
                nc.sync.dma_start(out=a2c_b_s[:], in_=a2c_b_d[:])
                onesNBL = w1pool.tile([1, NBL], BF16, tag="onesNBL",
                                      name="onesNBL")
                nc.vector.memset(onesNBL[:], 1.0)
                p_attT = [w1pool.tile([128, NBL], BF16, tag=f"pattT{hc}",
                                      name=f"pattT{hc}")
                          for hc in range(HCN)]
                stat_all = w1pool.tile([128, LP], FP8, tag="stat_all",
                                       name="stat_all")
                nc.vector.memset(stat_all[:], 0.0)
                w_f8 = w1pool.tile([BMY, LP], FP8, tag="w_f8", name="w_f8")
                nc.vector.memset(w_f8[:], 0.0)
                zh = w1pool.tile([128, 64], BF16, tag="zh", name="zh")
                nc.vector.memset(zh[:], 0.0)
                zf8 = w1pool.tile([128, 384], FP8, tag="zf8", name="zf8")
                nc.vector.memset(zf8[:], 0.0)
                c_st = w1pool.tile([B, GC], FP32, tag="c_st", name="c_st")

                def emit_rep(rep):
                    agA_out = agA_out_r[rep]
                    agH_out = agH_out_r[rep]
                    agS_out = agS_out_r[rep]

                    def probe(name, src_ap, shape, dt):
                        if rep == 0 and name in probes:
                            probe_(name, src_ap, shape, dt)

                    nc.vector.memset(c_st[:], 0.0)
                    # h2 blocks: [128, rc(8) x half(2) x b(64)]; block k
                    # holds h_{2k+1} (half 0) and h_{2k+2} (half 1), as 2*h.
                    # h2f8 blocks mirror them in fp8 for ah/logit matmuls.
                    h2b = [None] * NPAIR
                    h2f8b = [None] * NPAIR

                    def h_ap(j, rc):
                        """lhsT slice [128, 64] for h_j, R-chunk rc."""
                        if j == 0:
                            return zh[:]
                        blk = h2b[(j - 1) // 2]
                        c0 = rc * 128 + ((j - 1) % 2) * 64
                        return blk[:, c0:c0 + 64]

                    # ---------- phase 0 ----------
                    with (
                        tc.tile_pool(name=f"ctxpool{rep}", bufs=1) as ctxpool,
                        tc.tile_pool(name=f"stream{rep}", bufs=3) as stream,
                    ):
                        ctxT_s = load_chunks(ctxpool, ctxT_d, 2 * H, 8,
                                             "ctxT", FP8)
                        QW = 392
                        for q in range(4):
                            n0 = q * QW
                            _pa_tags = ["sums", "ah", "ar", "lg"]
                            pa_ps = [psum.tile([128, QW], FP32,
                                               tag=_pa_tags[hc],
                                               name=f"pa{hc}",
                                               bufs=(2 if hc == 3 else 1))
                                     for hc in range(HCN)]
                            for fc2 in range(8):
                                at = stream.tile([128, 2 * QW], FP8,
                                                 tag="attTq", name="attTq")
                                nc.sync.dma_start(
                                    out=at[:].rearrange(
                                        "p (two n) -> p two n", two=2),
                                    in_=attT_d[fc2 * 256:(fc2 + 1) * 256,
                                               n0:n0 + QW].rearrange(
                                        "(two p) n -> p two n", two=2))
                                for hc in range(HCN):
                                    lhs = AP(ctxT_s[fc2][:].tensor,
                                             ctxT_s[fc2][:].offset
                                             + hc * 128,
                                             [list(ctxT_s[fc2][:].ap[0]),
                                              [H, 2], [1, 128]])
                                    nc.tensor.matmul(
                                        pa_ps[hc][:], lhs,
                                        at[:].rearrange(
                                            "p (two n) -> p two n", two=2),
                                        start=(fc2 == 0), stop=False,
                                        perf_mode=PM.DoubleRow)
                            for hc in range(HCN):
                                nc.tensor.matmul(
                                    pa_ps[hc][:],
                                    ctx_b_s[:, hc * 128:(hc + 1) * 128],
                                    onesNBL[:, n0:n0 + QW], start=False,
                                    stop=True)
                                nc.vector.tensor_copy(
                                    p_attT[hc][:, n0:n0 + QW], pa_ps[hc][:])
                    probe("p_attT0", p_attT[0][:], [128, NBL], BF16)

                    def emit_logit_pair(k, chunks=(0, 512, 1024)):
                        """Logit matmul for step pair k from h2f8b[k]
                        (M=128), bias-add into lgb[k] (bf16)."""
                        fblk = h2f8b[k]
                        for ci, c0 in enumerate(chunks):
                            c1 = min(VP, c0 + 512)
                            lg_ps = psum.tile([128, 512], FP32, tag="lg",
                                              name="lg_ps", bufs=2)
                            for rc2 in range(4):
                                lhs = AP(fblk[:].tensor,
                                         fblk[:].offset + rc2 * 256,
                                         [list(fblk[:].ap[0]),
                                          [128, 2], [1, 128]])
                                rhs = AP(logitT_s[rc2][:].tensor,
                                         logitT_s[rc2][:].offset + c0,
                                         [list(logitT_s[rc2][:].ap[0]),
                                          [VP, 2], [1, c1 - c0]])
                                nc.tensor.matmul(
                                    lg_ps[:, 0:c1 - c0], lhs, rhs,
                                    start=(rc2 == 0), stop=(rc2 == 3),
                                    perf_mode=PM.DoubleRow)
                            nc.vector.scalar_tensor_tensor(
                                lgb[k][:, c0:c1], lg_ps[:, 0:c1 - c0],
                                1.0 / 8.0, logit_b_s[:, c0:c1],
                                op0=ALU.mult, op1=ALU.add)

                    def emit_pair_stats(k, work):
                        """Softmax stats for pair k (reads lgb[k], SBUF)."""
                        nc.vector.tensor_reduce(
                            negm_all[:, k:k + 1], lgb[k][:], axis=AX.X,
                            op=ALU.max, negate=True)
                        junk = work.tile([128, VP], BF16, tag="p2junk",
                                         name="p2junk", bufs=2)
                        nc.scalar.activation(
                            junk[:], lgb[k][:], AF.Exp,
                            bias=negm_all[:, k:k + 1],
                            accum_out=s_all[:, k:k + 1])

                    # ---------- phase 1 ----------
                    with tc.tile_pool(name=f"work1_{rep}", bufs=1) as work:
                        for t in range(t_steps):
                            # stats for the pair computed two steps ago run
                            # in this step's sums/ah window (DVE+ACT idle)
                            if t >= 3 and t % 2 == 1:
                                emit_pair_stats((t - 3) // 2, work)
                            # xt matmuls first: no h dependency, so they run
                            # inside the agH window and keep PE ramped
                            sums_ps = psum.tile([B, NGATE], FP32, tag="sums",
                                                name="sums", bufs=1)
                            for c0 in (0, 512):
                                c1 = min(NGATE, c0 + 512)
                                for kc in range(3):
                                    nc.tensor.matmul(
                                        sums_ps[:, c0:c1],
                                        xtT_s[kc][:, t * B:(t + 1) * B],
                                        i2hT_s[kc][:, c0:c1],
                                        start=(kc == 0), stop=False)
                            for c0 in (0, 512):
                                c1 = min(NGATE, c0 + 512)
                                for rc in range(RCN):
                                    nc.tensor.matmul(
                                        sums_ps[:, c0:c1],
                                        h_ap(t, rc),
                                        h2hT_s[rc][:, c0:c1],
                                        start=False, stop=(rc == RCN - 1))

                            ah_ps = psum.tile([B, H], FP32, tag="ah",
                                              name="ah", bufs=1)
                            for rc2 in range(4):
                                if t == 0:
                                    lhs = AP(zf8[:].tensor, zf8[:].offset,
                                             [list(zf8[:].ap[0]),
                                              [128, 2], [1, 64]])
                                else:
                                    fb = h2f8b[(t - 1) // 2]
                                    lhs = AP(fb[:].tensor,
                                             fb[:].offset + rc2 * 256
                                             + ((t - 1) % 2) * 64,
                                             [list(fb[:].ap[0]),
                                              [128, 2], [1, 64]])
                                rhs = AP(h2attT_s[rc2][:].tensor,
                                         h2attT_s[rc2][:].offset,
                                         [list(h2attT_s[rc2][:].ap[0]),
                                          [H, 2], [1, H]])
                                nc.tensor.matmul(ah_ps[:], lhs, rhs,
                                                 start=(rc2 == 0),
                                                 stop=False,
                                                 perf_mode=PM.DoubleRow)
                            nc.tensor.matmul(ah_ps[:], ones64[:],
                                             h2att_b_s[:], start=False,
                                             stop=True)
                            ah_sb = work.tile([B, H], BF16, tag="ah_sb",
                                              name="ah_sb", bufs=1)
                            nc.scalar.copy(ah_sb[:], ah_ps[:])
                            ahT_ps = psum.tile([128, HCN * 8], FP32,
                                               tag="small", name="ahT_ps",
                                               bufs=1)
                            for hc in range(HCN):
                                nc.tensor.matmul(
                                    ahT_ps[:, hc * 8:(hc + 1) * 8],
                                    ah_sb[:, hc * 128:(hc + 1) * 128],
                                    bsel_s[:], start=True, stop=True)
                            ahT = work.tile([128, HCN * 8], BF16,
                                            tag="ahT_sb", name="ahT_sb",
                                            bufs=1)
                            nc.vector.tensor_copy(ahT[:], ahT_ps[:])

                            e_ps = psum.tile([BMY, L], FP32, tag="small",
                                             name="e_ps", bufs=1)
                            for hcp in range(2):
                                dt2 = work.tile([128, 2 * NBL], FP8,
                                                tag="dt2", name="dt2",
                                                bufs=2)
                                for i in range(2):
                                    hc = 2 * hcp + i
                                    dp = work.tile([128, NBL], BF16,
                                                   tag="dp", name="dp",
                                                   bufs=2)
                                    eng = (nc.gpsimd if (hcp, i) == (1, 1)
                                           else nc.vector)
                                    eng.tensor_tensor(
                                        dp[:].rearrange("p (b l) -> p b l",
                                                        b=BMY),
                                        p_attT[hc][:].rearrange(
                                            "p (b l) -> p b l", b=BMY),
                                        bcast_free(
                                            ahT[:, hc * 8:(hc + 1) * 8], L),
                                        op=ALU.add)
                                    nc.scalar.activation(
                                        dt2[:, i * NBL:(i + 1) * NBL],
                                        dp[:], AF.Tanh)
                                for b in range(BMY):
                                    lhs = AP(alpha_s[hcp][:].tensor,
                                             alpha_s[hcp][:].offset + b * 32,
                                             [list(alpha_s[hcp][:].ap[0]),
                                              [16, 2], [1, BMY]])
                                    rhs = AP(dt2[:].tensor,
                                             dt2[:].offset + b * L,
                                             [list(dt2[:].ap[0]),
                                              [NBL, 2], [1, L]])
                                    nc.tensor.matmul(
                                        e_ps[:], lhs, rhs,
                                        start=(hcp == 0 and b == 0),
                                        stop=(hcp == 1 and b == BMY - 1),
                                        perf_mode=PM.DoubleRow)

                            # e_ps holds 32*e; |e| <~ 3 so exp(e) is safe
                            # without max-subtraction (alpha ~N(0,.02))
                            u = work.tile([BMY, L], FP32, tag="u", name="u",
                                          bufs=1)
                            ssum = work.tile([BMY, 1], FP32, tag="ssum",
                                             name="ssum", bufs=1)
                            nc.scalar.activation(u[:], e_ps[:], AF.Exp,
                                                 scale=1.0 / W_SCALE,
                                                 accum_out=ssum[:])
                            rinv = work.tile([BMY, 1], FP32, tag="rinv",
                                             name="rinv", bufs=1)
                            nc.vector.reciprocal(rinv[:], ssum[:])
                            nc.vector.tensor_scalar(w_f8[:, 0:L], u[:],
                                                    rinv[:], W_SCALE,
                                                    op0=ALU.mult,
                                                    op1=ALU.mult)

                            wdr = dpool.tile([BMY, LP], FP8, tag="wdr",
                                             name="wdr")
                            nc.sync.dma_start(out=wdr[:], in_=w_f8[:])
                            for b in range(BMY):
                                deng = nc.sync if b % 2 == 0 else nc.scalar
                                deng.dma_start(
                                    out=stat_all[b * 16:(b + 1) * 16,
                                                 b:LP:16],
                                    in_=wdr[b:b + 1, :].rearrange(
                                        "o (g lp) -> (o lp) g", g=NG))

                            # att_res: fp8 DoubleRow, 7 k-tile pairs
                            ar_sb = work.tile([BMY, F], BF16, tag="ar_sb",
                                              name="ar_sb", bufs=1)
                            for fq in range(4):
                                f0 = fq * 512
                                ar_ps = psum.tile([BMY, 512], FP32,
                                                  tag="ar", name="ar_ps",
                                                  bufs=1)
                                for q in range(NG // 2):
                                    lhs = AP(stat_all[:].tensor,
                                             stat_all[:].offset + q * 32,
                                             [list(stat_all[:].ap[0]),
                                              [16, 2], [1, BMY]])
                                    rhs = AP(att_f8_s[:].tensor,
                                             att_f8_s[:].offset
                                             + 2 * q * F + f0,
                                             [list(att_f8_s[:].ap[0]),
                                              [F, 2], [1, 512]])
                                    nc.tensor.matmul(
                                        ar_ps[:], lhs, rhs,
                                        start=(q == 0),
                                        stop=(q == NG // 2 - 1),
                                        perf_mode=PM.DoubleRow)
                                nc.scalar.copy(
                                    ar_sb[:, f0:f0 + 512], ar_ps[:])
                            agA_in = dpool.tile([BMY, F], BF16, tag="agA_in",
                                                name="agA_in")
                            nc.sync.dma_start(out=agA_in[:], in_=ar_sb[:])
                            if no_cc:
                                nc.sync.dma_start(out=agA_out[t][0:BMY, :],
                                                  in_=agA_in[:])
                            else:
                                nc.gpsimd.collective_compute(
                                    "AllGather", ALU.bypass,
                                    replica_groups=RG,
                                    ins=[agA_in.opt()], outs=[agA_out[t][:]])

                            # pre-AG gate work runs in the AllGather window
                            sig3 = work.tile([B, 384], FP32, tag="sig3",
                                             name="sig3", bufs=1)
                            nc.scalar.activation(sig3[:], sums_ps[:, 0:384],
                                                 AF.Tanh, scale=0.5)
                            sitr = work.tile([B, 256], FP32, tag="sitr",
                                             name="sitr", bufs=1)
                            nc.scalar.copy(sitr[:], sums_ps[:, 384:640])
                            a_t = work.tile([B, GC], FP32, tag="a_t",
                                            name="a_t", bufs=1)
                            nc.vector.scalar_tensor_tensor(
                                a_t[:], sig3[:, 128:256], 1.0, c_st[:],
                                op0=ALU.add, op1=ALU.mult)

                            # paired logit for block (t-2)//2 runs in the
                            # AllGather window (last chunk lands in the agH
                            # window below)
                            if t >= 2 and t % 2 == 0:
                                emit_logit_pair((t - 2) // 2, (0, 512))

                            arg_sb = work.tile([B, F], BF16, tag="arg_sb",
                                               name="arg_sb", bufs=1)
                            nc.sync.dma_start(out=arg_sb[:],
                                              in_=agA_out[t][:])
                            arT = work.tile([128, FCN * 64], FP8, tag="arT",
                                            name="arT", bufs=1)
                            for fc in range(FCN):
                                art_ps = psum.tile(
                                    [128, 64], BF16,
                                    tag=("small" if fc % 2 else "ctx"),
                                    name="art_ps", bufs=1)
                                nc.tensor.transpose(
                                    art_ps[:],
                                    arg_sb[:, fc * 128:(fc + 1) * 128],
                                    ident_s[0:B, 0:B])
                                ceng = nc.vector if fc % 2 else nc.scalar
                                if ceng is nc.vector:
                                    ceng.tensor_copy(
                                        arT[:, fc * 64:(fc + 1) * 64],
                                        art_ps[:])
                                else:
                                    ceng.copy(
                                        arT[:, fc * 64:(fc + 1) * 64],
                                        art_ps[:])

                            # ctx_ps holds 512*ctx (32 from w, 16 from a2c)
                            ctx_ps = psum.tile([B, 256], FP32, tag="ctx",
                                               name="ctx_ps", bufs=1)
                            for fc2 in range(8):
                                lhs = AP(arT[:].tensor,
                                         arT[:].offset + fc2 * 128,
                                         [list(arT[:].ap[0]),
                                          [64, 2], [1, 64]])
                                rhs = AP(a2cT_s[fc2][:].tensor,
                                         a2cT_s[fc2][:].offset,
                                         [list(a2cT_s[fc2][:].ap[0]),
                                          [256, 2], [1, 256]])
                                nc.tensor.matmul(
                                    ctx_ps[:], lhs, rhs, start=(fc2 == 0),
                                    stop=False, perf_mode=PM.DoubleRow)
                            nc.tensor.matmul(ctx_ps[:], ones64[:],
                                             a2c_b_s[:], start=False,
                                             stop=True)

                            itr1 = work.tile([B, GC], FP32, tag="itr1",
                                             name="itr1", bufs=1)
                            nc.vector.scalar_tensor_tensor(
                                itr1[:], ctx_ps[:, 0:128], 1.0 / 512.0,
                                sitr[:, 0:128], op0=ALU.mult, op1=ALU.add)
                            itr2 = work.tile([B, GC], FP32, tag="itr2",
                                             name="itr2", bufs=1)
                            nc.vector.scalar_tensor_tensor(
                                itr2[:], ctx_ps[:, 128:256], 1.0 / 512.0,
                                sitr[:, 128:256], op0=ALU.mult, op1=ALU.add)
                            g_t = work.tile([B, GC], FP32, tag="g_t",
                                            name="g_t", bufs=1)
                            nc.vector.tensor_tensor(g_t[:], itr1[:],
                                                    itr2[:], op=ALU.max)
                            b_t = work.tile([B, GC], FP32, tag="b_t",
                                            name="b_t", bufs=1)
                            nc.vector.scalar_tensor_tensor(
                                b_t[:], sig3[:, 0:128], 1.0, g_t[:],
                                op0=ALU.add, op1=ALU.mult)
                            nc2_t = work.tile([B, GC], FP32, tag="nc2",
                                              name="nc2", bufs=1)
                            nc.vector.tensor_tensor(nc2_t[:], a_t[:],
                                                    b_t[:], op=ALU.add)
                            nc.vector.tensor_scalar(c_st[:], nc2_t[:], 0.5,
                                                    None, op0=ALU.mult)
                            tnc = work.tile([B, GC], FP32, tag="tnc",
                                            name="tnc", bufs=1)
                            nc.scalar.activation(tnc[:], nc2_t[:], AF.Tanh,
                                                 scale=0.5)
                            nh2 = work.tile([B, GC], BF16, tag="nh2",
                                            name="nh2", bufs=1)
                            nc.vector.scalar_tensor_tensor(
                                nh2[:], sig3[:, 256:384], 1.0, tnc[:],
                                op0=ALU.add, op1=ALU.mult)

                            nhT_ps = psum.tile([GC, B], BF16, tag="small",
                                               name="nhT_ps", bufs=1)
                            nc.tensor.transpose(nhT_ps[:], nh2[:],
                                                ident_s[0:B, 0:B])
                            nhT_sb = work.tile([GC, B], BF16, tag="nhT_sb",
                                               name="nhT_sb", bufs=1)
                            nc.vector.tensor_copy(nhT_sb[:], nhT_ps[:])
                            agH_in = dpool.tile([GC, B], BF16, tag="agH_in",
                                                name="agH_in")
                            nc.sync.dma_start(out=agH_in[:], in_=nhT_sb[:])
                            if no_cc:
                                nc.sync.dma_start(out=agH_out[t][0:GC, :],
                                                  in_=agH_in[:])
                            else:
                                nc.gpsimd.collective_compute(
                                    "AllGather", ALU.bypass,
                                    replica_groups=RG,
                                    ins=[agH_in.opt()], outs=[agH_out[t][:]])
                            if t >= 2 and t % 2 == 0:
                                emit_logit_pair((t - 2) // 2, (1024,))
                            # h_{t+1} -> block t//2, half t%2
                            if t % 2 == 0:
                                h2b[t // 2] = hpool.tile(
                                    [128, RCN * 128], BF16, tag="h2",
                                    name=f"h2_{t // 2}")
                                h2f8b[t // 2] = hpool.tile(
                                    [128, RCN * 128], FP8, tag="h2f8",
                                    name=f"h2f8_{t // 2}")
                            blk = h2b[t // 2]
                            dst = AP(blk[:].tensor,
                                     blk[:].offset + (t % 2) * 64,
                                     [list(blk[:].ap[0]), [128, RCN],
                                      [1, 64]])
                            nc.sync.dma_start(
                                out=dst,
                                in_=agH_out[t][:].rearrange(
                                    "(rc rl) b -> rl rc b", rc=RCN))
                            fblk = h2f8b[t // 2]
                            fsrc = AP(blk[:].tensor,
                                      blk[:].offset + (t % 2) * 64,
                                      [list(blk[:].ap[0]), [128, RCN],
                                       [1, 64]])
                            fdst = AP(fblk[:].tensor,
                                      fblk[:].offset + (t % 2) * 64,
                                      [list(fblk[:].ap[0]), [128, RCN],
                                       [1, 64]])
                            nc.vector.tensor_copy(fdst, fsrc)

                            if t == 0:
                                probe("ah0", ah_sb[:], [B, H], BF16)
                                probe("ahT0", ahT[:], [128, HCN * 8], BF16)
                                probe("u0", u[:], [BMY, L], FP32)
                                probe("statall0", stat_all[:],
                                      [128, LP], FP8)
                                probe("ar0", ar_sb[:], [BMY, F], BF16)
                                probe("arT0", arT[:], [128, FCN * 64], BF16)
                                probe("nh20", nh2[:], [B, GC], BF16)
                                probe("agH0", agH_out[0][:], [R, B], BF16)

                        # remaining pairs + stats
                        emit_logit_pair(NPAIR - 1)
                        emit_pair_stats(NPAIR - 1, work)
                        probe("lgb0", lgb[0][:], [128, VP], BF16)

                    # ---------- phase 2 ----------
                    with tc.tile_pool(name=f"p2_{rep}", bufs=2) as p2:
                        agS_in = dpool.tile([128, 2 * NT], FP32,
                                            tag="agS_in", name="agS_in")
                        nc.sync.dma_start(out=agS_in[:, 0:NT],
                                          in_=negm_all[:])
                        nc.sync.dma_start(out=agS_in[:, NT:2 * NT],
                                          in_=s_all[:])
                        if no_cc:
                            nc.sync.dma_start(out=agS_out[0:128, :],
                                              in_=agS_in[:])
                        else:
                            nc.gpsimd.collective_compute(
                                "AllGather", ALU.bypass, replica_groups=RG,
                                ins=[agS_in.opt()], outs=[agS_out[:]])
                        statg = p2.tile([128, NC * 2 * NT], FP32,
                                        tag="statg", name="statg", bufs=1)
                        nc.sync.dma_start(
                            out=statg[:].rearrange("p (r s) -> p r s", r=NC),
                            in_=agS_out[:].rearrange("(r p) s -> p r s",
                                                     r=NC))
                        sview = statg[:].rearrange("p (r s) -> p s r", r=NC)
                        negM = p2.tile([128, NT], FP32, tag="negM",
                                       name="negM", bufs=1)
                        nc.vector.tensor_reduce(negM[:], sview[:, 0:NT, :],
                                                axis=AX.X, op=ALU.min)
                        earg = p2.tile([128, NT * NC], FP32, tag="earg",
                                       name="earg", bufs=1)
                        nc.vector.tensor_tensor(
                            earg[:].rearrange("p (s r) -> p s r", r=NC),
                            bcast_free(negM[:], NC), sview[:, 0:NT, :],
                            op=ALU.subtract)
                        em = p2.tile([128, NT * NC], FP32, tag="em",
                                     name="em", bufs=1)
                        nc.scalar.activation(em[:], earg[:], AF.Exp)
                        sexp = p2.tile([128, NT * NC], FP32, tag="sexp",
                                       name="sexp", bufs=1)
                        nc.vector.tensor_tensor(
                            sexp[:].rearrange("p (s r) -> p s r", r=NC),
                            em[:].rearrange("p (s r) -> p s r", r=NC),
                            sview[:, NT:2 * NT, :], op=ALU.mult)
                        S_t = p2.tile([128, NT], FP32, tag="S_t",
                                      name="S_t", bufs=1)
                        nc.vector.tensor_reduce(
                            S_t[:],
                            sexp[:].rearrange("p (s r) -> p s r", r=NC),
                            axis=AX.X, op=ALU.add)
                        lnS = p2.tile([128, NT], FP32, tag="lnS",
                                      name="lnS", bufs=1)
                        nc.scalar.activation(lnS[:], S_t[:], AF.Ln)
                        logZ = p2.tile([128, NT], FP32, tag="logZ",
                                       name="logZ", bufs=1)
                        nc.vector.scalar_tensor_tensor(
                            logZ[:], negM[:], -1.0, lnS[:], op0=ALU.mult,
                            op1=ALU.add)
                        probe("logZ", logZ[:], [128, NT], FP32)
                        for tt in range(NT):
                            lp_t = p2.tile([128, VP], FP32, tag="p2lp",
                                           name="p2lp", bufs=4)
                            eng = nc.vector if tt % 2 == 0 else nc.gpsimd
                            eng.tensor_scalar(
                                lp_t[:], lgb[tt][:], logZ[:, tt:tt + 1],
                                None, op0=ALU.subtract)
                            nc.sync.dma_start(
                                out=out_d[tt * 128:(tt + 1) * 128, :],
                                in_=lp_t[:])

                for rep in range(reps):
                    emit_rep(rep)

    nc.compile()
    return nc, sorted(probes)


_NC_CACHE = {}


def kernel(**inputs):
    """Full-input entry point: returns logp [B, T, V1] float32."""
    from concourse.bass_utils import run_bass_kernel_spmd
    in_maps = host_prep(inputs)
    if "nc" not in _NC_CACHE:
        _NC_CACHE["nc"], _ = build(T, (), reps=1)
    nc = _NC_CACHE["nc"]
    res = run_bass_kernel_spmd(nc, in_maps, list(range(NC)))
    outs = [res.results[c]["logp"] for c in range(NC)]
    full = np.concatenate(outs, axis=1)[:, :V1]          # [T*B, V1]
    logp = full.reshape(T, B, V1).transpose(1, 0, 2)
    return np.ascontiguousarray(logp.astype(np.float32))


# revision 50
# speedup vs baseline: 1.2896x; 1.1782x over previous
"""Attention-LSTM captioning model on 8 trn2 cores (8-way tensor parallel).

Gate/itr/vocab output dims sharded across cores (full B=64 per core);
attention batch-sharded (8 batches/core, selected via per-core one-hot bsel
data, since the SPMD program is identical on every core). Activations are
transposed [feature, batch]. Per step: AllGather(att_res), AllGather(nh
chunk). Sigmoid(x) is computed as (tanh(x/2)+1)/2 so phase 1 only needs the
{tanh, exp} ACT table; the hidden state is stored as 2*h with h-consuming
weights pre-halved on the host.

v2: att_res matmul runs in fp8e4 DoubleRow mode (w scaled x32, a2c
pre-divided); the logit matmul is computed per step-pair with M=128 (both
steps' h in one stationary tile), scheduled one pair late so it lands in
the att_res AllGather window; logits stay in SBUF as bf16 and log_softmax
stats are folded into phase 1, so phase 2 is just one stats AllGather and
the final normalize (no DRAM scratch roundtrip).
"""
import numpy as np
import ml_dtypes

import concourse.bacc as bacc
import concourse.mybir as mybir
import concourse.tile as tile
from concourse.ap import AP
from concourse.bass_utils import run_bass_kernel_spmd

BF16_NP = ml_dtypes.bfloat16
FP8_NP = ml_dtypes.float8_e4m3
FP32 = mybir.dt.float32
BF16 = mybir.dt.bfloat16
FP8 = mybir.dt.float8e4
AF = mybir.ActivationFunctionType
ALU = mybir.AluOpType
AX = mybir.AxisListType
PM = mybir.MatmulPerfMode

B, T, R, H, F, E, L, V1 = 64, 20, 1024, 512, 2048, 300, 196, 12001
NC = 8
BMY = B // NC
GC = R // NC              # 128
NGATE = 5 * GC            # 640
VP = 1504
NG = 14                   # l-groups of 16 (224 >= L), even for fp8 pairs
LP = NG * 16              # 224
W_SCALE = 32.0
EP = 384
HCN = H // 128            # 4
FCN = F // 128            # 16
RCN = R // 128            # 8
NBL = BMY * L             # 1568


def _bf(x):
    return np.ascontiguousarray(np.asarray(x, dtype=np.float32)).astype(BF16_NP)


def _f8(x):
    return np.ascontiguousarray(np.asarray(x, dtype=np.float32)).astype(FP8_NP)


def bcast_free(ap, n):
    """Append a step-0 free dim of size n to an AP (broadcast)."""
    return AP(ap.tensor, ap.offset, list(ap.ap) + [[0, n]])


def host_prep(inputs):
    seq = np.asarray(inputs["seq"])
    att = np.asarray(inputs["att_feats"], dtype=np.float32)
    embed_w = np.asarray(inputs["embed_w"], dtype=np.float32)
    ctx2att_w = np.asarray(inputs["ctx2att_w"], dtype=np.float32)
    ctx2att_b = np.asarray(inputs["ctx2att_b"], dtype=np.float32)
    h2att_w = np.asarray(inputs["h2att_w"], dtype=np.float32)
    h2att_b = np.asarray(inputs["h2att_b"], dtype=np.float32)
    alpha_w = np.asarray(inputs["alpha_w"], dtype=np.float32)
    i2h_w = np.asarray(inputs["i2h_w"], dtype=np.float32)
    i2h_b = np.asarray(inputs["i2h_b"], dtype=np.float32)
    h2h_w = np.asarray(inputs["h2h_w"], dtype=np.float32)
    h2h_b = np.asarray(inputs["h2h_b"], dtype=np.float32)
    a2c_w = np.asarray(inputs["a2c_w"], dtype=np.float32)
    a2c_b = np.asarray(inputs["a2c_b"], dtype=np.float32)
    logit_w = np.asarray(inputs["logit_w"], dtype=np.float32)
    logit_b = np.asarray(inputs["logit_b"], dtype=np.float32)

    xt = embed_w[seq]                                    # [B, T, E]
    xtT = np.zeros((EP, T * B), dtype=np.float32)
    xtT[:E] = xt.transpose(2, 1, 0).reshape(E, T * B)
    xtT[E] = 1.0
    xtT = _bf(xtT)
    bias_gate = i2h_b + h2h_b

    in_maps = []
    for c in range(NC):
        m = {"xtT": xtT}
        grows = np.concatenate([np.arange(gg * R + c * GC, gg * R + (c + 1) * GC)
                                for gg in range(5)])
        i2hT = np.zeros((EP, NGATE), dtype=np.float32)
        i2hT[:E] = i2h_w[grows, :].T
        i2hT[E] = bias_gate[grows]
        m["i2hT"] = _bf(i2hT)
        m["h2hT"] = _bf(h2h_w[grows, :].T * 0.5)
        ht8 = (h2att_w.T * 0.5).reshape(4, 2, 128, H)
        m["h2attT_f8"] = _f8(ht8.transpose(0, 2, 1, 3).reshape(512, 2 * H))
        m["h2att_bias"] = _bf(h2att_b[None, :])
        m["ctxT"] = _bf(ctx2att_w.T)
        m["ctx_bias"] = _bf(ctx2att_b[None, :])
        amy = att[c * BMY:(c + 1) * BMY]                 # [8, L, F]
        m["attT_f8"] = _f8(amy.transpose(2, 0, 1).reshape(F, NBL))
        # ctx2att in fp8 k-tile-pair layout: [pair*128+p, i*H+h] =
        # ctx2att_w.T[pair*256+i*128+p, h]
        cT = ctx2att_w.T.reshape(8, 2, 128, H)           # [pair, i, p, h]
        m["ctxT_f8"] = _f8(cT.transpose(0, 2, 1, 3).reshape(1024, 2 * H))
        # fp8 att for the att_res matmul: part p=(b*16+lg), col=(g*F+f),
        # value att[b, g*16+lg, f]
        apad = np.zeros((BMY, LP, F), dtype=np.float32)
        apad[:, :L] = amy
        m["att_f8"] = _f8(apad.reshape(BMY, NG, 16, F).transpose(0, 2, 1, 3)
                          .reshape(128, NG * F))
        # alpha one-hot diag, fp8 k-pair layout per hc-pair:
        # [hcp*128+p, b*32+i*16+m] = 32*alpha[(2hcp+i)*128+p] iff m==b
        ac = np.zeros((2, 128, BMY, 2, 16), dtype=np.float32)
        for hcp in range(2):
            for i in range(2):
                for b in range(BMY):
                    ac[hcp, :, b, i, b] = \
                        W_SCALE * alpha_w[0, (2 * hcp + i) * 128:
                                          (2 * hcp + i + 1) * 128]
        m["alpha_f8"] = _f8(ac.reshape(256, BMY * 32))
        arows = np.concatenate([np.arange(c * GC, (c + 1) * GC),
                                np.arange(R + c * GC, R + (c + 1) * GC)])
        # a2c in fp8 k-pair layout, scaled x16 (itr add divides by 512)
        a2cp = (a2c_w[arows, :].T * 16.0).reshape(8, 2, 128, 256)
        m["a2cT_f8"] = _f8(a2cp.transpose(0, 2, 1, 3).reshape(1024, 512))
        m["a2c_bias"] = _bf(a2c_b[arows][None, :] * 512.0)
        vrows = np.arange(c * VP, (c + 1) * VP)
        lw = np.zeros((R, VP), dtype=np.float32)
        lb = np.full((1, VP), -1e30, dtype=np.float32)
        valid = vrows < V1
        lw[:, valid] = logit_w[vrows[valid], :].T * 0.5
        lb[0, valid] = logit_b[vrows[valid]]
        # fp8 k-pair layout, x8 for fp8 range (bias-add divides by 8)
        lwp = (lw * 8.0).reshape(4, 2, 128, VP)
        m["logitT_f8"] = _f8(lwp.transpose(0, 2, 1, 3).reshape(512, 2 * VP))
        m["logit_bias"] = lb
        m["ident"] = _bf(np.eye(128))
        bsel = np.zeros((B, BMY), dtype=np.float32)
        for j in range(BMY):
            bsel[c * BMY + j, j] = 1.0
        m["bsel"] = _bf(bsel)
        in_maps.append(m)
    return in_maps


def build(t_steps=T, probes=(), reps=1, no_cc=False):
    assert t_steps % 2 == 0
    nc = bacc.Bacc("TRN2", target_bir_lowering=False, debug=False,
                   num_devices=NC)
    probes = set(probes)
    NT = t_steps * B // 128
    NPAIR = t_steps // 2
    RG = [list(range(NC))]

    def din(name, shape, dt=BF16):
        return nc.dram_tensor(name, shape, dt, kind="ExternalInput")

    xtT_d = din("xtT", [EP, T * B])
    i2hT_d = din("i2hT", [EP, NGATE])
    h2hT_d = din("h2hT", [R, NGATE])
    h2attT_d = din("h2attT_f8", [512, 2 * H], FP8)
    h2att_b_d = din("h2att_bias", [1, H])
    ctxT_d = din("ctxT_f8", [1024, 2 * H], FP8)
    ctx_b_d = din("ctx_bias", [1, H])
    attT_d = din("attT_f8", [F, NBL], FP8)
    att_f8_d = din("att_f8", [128, NG * F], FP8)
    alpha_d = din("alpha_f8", [256, BMY * 32], FP8)
    a2cT_d = din("a2cT_f8", [1024, 512], FP8)
    a2c_b_d = din("a2c_bias", [1, 256])
    logitT_d = din("logitT_f8", [512, 2 * VP], FP8)
    logit_b_d = din("logit_bias", [1, VP], FP32)
    ident_d = din("ident", [128, 128])
    bsel_d = din("bsel", [B, BMY])

    out_d = nc.dram_tensor("logp", [t_steps * B, VP], FP32,
                           kind="ExternalOutput")
    agA_out_r = [[nc.dram_tensor(f"agA_out_{rp}_{t}", [B, F], BF16,
                                 addr_space="Shared") for t in range(t_steps)]
                 for rp in range(reps)]
    agH_out_r = [[nc.dram_tensor(f"agH_out_{rp}_{t}", [R, B], BF16,
                                 addr_space="Shared") for t in range(t_steps)]
                 for rp in range(reps)]
    agS_out_r = [nc.dram_tensor(f"agS_out_{rp}", [NC * 128, 2 * NT], FP32,
                                addr_space="Shared") for rp in range(reps)]

    with tile.TileContext(nc) as tc:
        with (
            tc.tile_pool(name="wpool", bufs=1) as wpool,
            tc.tile_pool(name="hpool", bufs=3) as hpool,
            tc.tile_pool(name="psum", bufs=1, space="PSUM") as psum,
            tc.tile_pool(name="dram", bufs=4, space="DRAM") as dpool,
        ):
            def probe_(name, src_ap, shape, dt):
                pd = nc.dram_tensor(f"probe_{name}", list(shape), dt,
                                    kind="ExternalOutput")
                nc.sync.dma_start(out=pd[:], in_=src_ap)

            def load_chunks(pool, dram, cols, n, tag, dt=BF16):
                ts = []
                for i in range(n):
                    t_ = pool.tile([128, cols], dt, tag=f"{tag}{i}",
                                   name=f"{tag}{i}")
                    nc.sync.dma_start(out=t_[:],
                                      in_=dram[i * 128:(i + 1) * 128, :])
                    ts.append(t_)
                return ts

            logitT_s = load_chunks(wpool, logitT_d, 2 * VP, 4, "logitT", FP8)
            logit_b_s = wpool.tile([128, VP], FP32, tag="logitb",
                                   name="logitb")
            _lb_src = AP(logit_b_d[:].tensor, logit_b_d[:].offset,
                         [[0, 128], [1, VP]])
            nc.sync.dma_start(out=logit_b_s[:], in_=_lb_src)
            ident_s = wpool.tile([128, 128], BF16, tag="ident", name="ident")
            nc.sync.dma_start(out=ident_s[:], in_=ident_d[:])
            ones64 = wpool.tile([1, B], BF16, tag="ones64", name="ones64")
            nc.vector.memset(ones64[:], 1.0)
            negm_all = wpool.tile([128, NT], FP32, tag="negm_all",
                                  name="negm_all")
            s_all = wpool.tile([128, NT], FP32, tag="s_all", name="s_all")
            # bf16 logits resident in SBUF, one tile per step-pair
            lgb = [wpool.tile([128, VP], BF16, tag=f"lgb{k}", name=f"lgb{k}")
                   for k in range(NPAIR)]

            with tc.tile_pool(name="w1pool", bufs=1) as w1pool:
                xtT_s = load_chunks(w1pool, xtT_d, T * B, 3, "xtT")
                i2hT_s = load_chunks(w1pool, i2hT_d, NGATE, 3, "i2hT")
                h2hT_s = load_chunks(w1pool, h2hT_d, NGATE, RCN, "h2hT")
                h2attT_s = load_chunks(w1pool, h2attT_d, 2 * H, 4,
                                       "h2attT", FP8)
                att_f8_s = w1pool.tile([128, NG * F], FP8, tag="attf8",
                                       name="attf8")
                nc.sync.dma_start(out=att_f8_s[:], in_=att_f8_d[:])
                alpha_s = load_chunks(w1pool, alpha_d, BMY * 32, 2,
                                      "alpha", FP8)
                a2cT_s = load_chunks(w1pool, a2cT_d, 512, 8, "a2cT", FP8)
                bsel_s = w1pool.tile([B, BMY], BF16, tag="bsel", name="bsel")
                nc.sync.dma_start(out=bsel_s[:], in_=bsel_d[:])
                h2att_b_s = w1pool.tile([1, H], BF16, tag="h2attb",
                                        name="h2attb")
                nc.sync.dma_start(out=h2att_b_s[:], in_=h2att_b_d[:])
                ctx_b_s = w1pool.tile([1, H], BF16, tag="ctxb", name="ctxb")
                nc.sync.dma_start(out=ctx_b_s[:], in_=ctx_b_d[:])
                a2c_b_s = w1pool.tile([1, 256], BF16, tag="a2cb", name="a2cb")
                nc.sync.dma_start(out=a2c_b_s[:], in_=a2c_b_d[:])
                onesNBL = w1pool.tile([1, NBL], BF16, tag="onesNBL",
                                      name="onesNBL")
                nc.vector.memset(onesNBL[:], 1.0)
                p_attT = [w1pool.tile([128, NBL], BF16, tag=f"pattT{hc}",
                                      name=f"pattT{hc}")
                          for hc in range(HCN)]
                stat_all = w1pool.tile([128, LP], FP8, tag="stat_all",
                                       name="stat_all")
                nc.vector.memset(stat_all[:], 0.0)
                w_f8 = w1pool.tile([BMY, LP], FP8, tag="w_f8", name="w_f8")
                nc.vector.memset(w_f8[:], 0.0)
                zh = w1pool.tile([128, 64], BF16, tag="zh", name="zh")
                nc.vector.memset(zh[:], 0.0)
                zf8 = w1pool.tile([128, 384], FP8, tag="zf8", name="zf8")
                nc.vector.memset(zf8[:], 0.0)
                c_st = w1pool.tile([B, GC], FP32, tag="c_st", name="c_st")

                def emit_rep(rep):
                    agA_out = agA_out_r[rep]
                    agH_out = agH_out_r[rep]
                    agS_out = agS_out_r[rep]

                    def probe(name, src_ap, shape, dt):
                        if rep == 0 and name in probes:
                            probe_(name, src_ap, shape, dt)

                    nc.vector.memset(c_st[:], 0.0)
                    # h2 blocks: [128, rc(8) x half(2) x b(64)]; block k
                    # holds h_{2k+1} (half 0) and h_{2k+2} (half 1), as 2*h.
                    # h2f8 blocks mirror them in fp8 for ah/logit matmuls.
                    h2b = [None] * NPAIR
                    h2f8b = [None] * NPAIR

                    def h_ap(j, rc):
                        """lhsT slice [128, 64] for h_j, R-chunk rc."""
                        if j == 0:
                            return zh[:]
                        blk = h2b[(j - 1) // 2]
                        c0 = rc * 128 + ((j - 1) % 2) * 64
                        return blk[:, c0:c0 + 64]

                    # ---------- phase 0 ----------
                    with tc.tile_pool(name=f"ctxpool{rep}", bufs=1) \
                            as ctxpool:
                        ctxT_s = load_chunks(ctxpool, ctxT_d, 2 * H, 8,
                                             "ctxT", FP8)
                        # preload all att k-pair tiles with 8 big DMAs
                        # split across the SP and ACT HWDGE queues
                        at2_s = []
                        for fc2 in range(8):
                            t_ = ctxpool.tile([128, 2 * NBL], FP8,
                                              tag=f"at2_{fc2}",
                                              name=f"at2_{fc2}")
                            deng = nc.sync if fc2 % 2 == 0 else nc.scalar
                            deng.dma_start(
                                out=t_[:].rearrange(
                                    "p (two n) -> p two n", two=2),
                                in_=attT_d[fc2 * 256:(fc2 + 1) * 256,
                                           :].rearrange(
                                    "(two p) n -> p two n", two=2))
                            at2_s.append(t_)
                        QW = 392
                        for q in range(4):
                            n0 = q * QW
                            _pa_tags = ["sums", "ah", "ar", "lg"]
                            pa_ps = [psum.tile([128, QW], FP32,
                                               tag=_pa_tags[hc],
                                               name=f"pa{hc}",
                                               bufs=(2 if hc == 3 else 1))
                                     for hc in range(HCN)]
                            for fc2 in range(8):
                                rhs = AP(at2_s[fc2][:].tensor,
                                         at2_s[fc2][:].offset + n0,
                                         [list(at2_s[fc2][:].ap[0]),
                                          [NBL, 2], [1, QW]])
                                for hc in range(HCN):
                                    lhs = AP(ctxT_s[fc2][:].tensor,
                                             ctxT_s[fc2][:].offset
                                             + hc * 128,
                                             [list(ctxT_s[fc2][:].ap[0]),
                                              [H, 2], [1, 128]])
                                    nc.tensor.matmul(
                                        pa_ps[hc][:], lhs, rhs,
                                        start=(fc2 == 0), stop=False,
                                        perf_mode=PM.DoubleRow)
                            for hc in range(HCN):
                                nc.tensor.matmul(
                                    pa_ps[hc][:],
                                    ctx_b_s[:, hc * 128:(hc + 1) * 128],
                                    onesNBL[:, n0:n0 + QW], start=False,
                                    stop=True)
                                nc.vector.tensor_copy(
                                    p_attT[hc][:, n0:n0 + QW], pa_ps[hc][:])
                    probe("p_attT0", p_attT[0][:], [128, NBL], BF16)

                    def emit_logit_pair(k, chunks=(0, 512, 1024)):
                        """Logit matmul for step pair k from h2f8b[k]
                        (M=128), bias-add into lgb[k] (bf16)."""
                        fblk = h2f8b[k]
                        for ci, c0 in enumerate(chunks):
                            c1 = min(VP, c0 + 512)
                            lg_ps = psum.tile([128, 512], FP32, tag="lg",
                                              name="lg_ps", bufs=2)
                            for rc2 in range(4):
                                lhs = AP(fblk[:].tensor,
                                         fblk[:].offset + rc2 * 256,
                                         [list(fblk[:].ap[0]),
                                          [128, 2], [1, 128]])
                                rhs = AP(logitT_s[rc2][:].tensor,
                                         logitT_s[rc2][:].offset + c0,
                                         [list(logitT_s[rc2][:].ap[0]),
                                          [VP, 2], [1, c1 - c0]])
                                nc.tensor.matmul(
                                    lg_ps[:, 0:c1 - c0], lhs, rhs,
                                    start=(rc2 == 0), stop=(rc2 == 3),
                                    perf_mode=PM.DoubleRow)
                            nc.vector.scalar_tensor_tensor(
                                lgb[k][:, c0:c1], lg_ps[:, 0:c1 - c0],
                                1.0 / 8.0, logit_b_s[:, c0:c1],
                                op0=ALU.mult, op1=ALU.add)

                    def emit_pair_stats(k, work):
                        """Softmax stats for pair k (reads lgb[k], SBUF)."""
                        nc.vector.tensor_reduce(
                            negm_all[:, k:k + 1], lgb[k][:], axis=AX.X,
                            op=ALU.max, negate=True)
                        junk = work.tile([128, VP], BF16, tag="p2junk",
                                         name="p2junk", bufs=2)
                        nc.scalar.activation(
                            junk[:], lgb[k][:], AF.Exp,
                            bias=negm_all[:, k:k + 1],
                            accum_out=s_all[:, k:k + 1])

                    # ---------- phase 1 ----------
                    with tc.tile_pool(name=f"work1_{rep}", bufs=1) as work:
                        for t in range(t_steps):
                            # stats for the pair computed two steps ago run
                            # in this step's sums/ah window (DVE+ACT idle)
                            if t >= 3 and t % 2 == 1:
                                emit_pair_stats((t - 3) // 2, work)
                            # xt matmuls first: no h dependency, so they run
                            # inside the agH window and keep PE ramped
                            sums_ps = psum.tile([B, NGATE], FP32, tag="sums",
                                                name="sums", bufs=1)
                            for c0 in (0, 512):
                                c1 = min(NGATE, c0 + 512)
                                for kc in range(3):
                                    nc.tensor.matmul(
                                        sums_ps[:, c0:c1],
                                        xtT_s[kc][:, t * B:(t + 1) * B],
                                        i2hT_s[kc][:, c0:c1],
                                        start=(kc == 0), stop=False)
                            for c0 in (0, 512):
                                c1 = min(NGATE, c0 + 512)
                                for rc in range(RCN):
                                    nc.tensor.matmul(
                                        sums_ps[:, c0:c1],
                                        h_ap(t, rc),
                                        h2hT_s[rc][:, c0:c1],
                                        start=False, stop=(rc == RCN - 1))

                            ah_ps = psum.tile([B, H], FP32, tag="ah",
                                              name="ah", bufs=1)
                            for rc2 in range(4):
                                if t == 0:
                                    lhs = AP(zf8[:].tensor, zf8[:].offset,
                                             [list(zf8[:].ap[0]),
                                              [128, 2], [1, 64]])
                                else:
                                    fb = h2f8b[(t - 1) // 2]
                                    lhs = AP(fb[:].tensor,
                                             fb[:].offset + rc2 * 256
                                             + ((t - 1) % 2) * 64,
                                             [list(fb[:].ap[0]),
                                              [128, 2], [1, 64]])
                                rhs = AP(h2attT_s[rc2][:].tensor,
                                         h2attT_s[rc2][:].offset,
                                         [list(h2attT_s[rc2][:].ap[0]),
                                          [H, 2], [1, H]])
                                nc.tensor.matmul(ah_ps[:], lhs, rhs,
                                                 start=(rc2 == 0),
                                                 stop=False,
                                                 perf_mode=PM.DoubleRow)
                            nc.tensor.matmul(ah_ps[:], ones64[:],
                                             h2att_b_s[:], start=False,
                                             stop=True)
                            ah_sb = work.tile([B, H], BF16, tag="ah_sb",
                                              name="ah_sb", bufs=1)
                            nc.scalar.copy(ah_sb[:], ah_ps[:])
                            ahT_ps = psum.tile([128, HCN * 8], FP32,
                                               tag="small", name="ahT_ps",
                                               bufs=1)
                            for hc in range(HCN):
                                nc.tensor.matmul(
                                    ahT_ps[:, hc * 8:(hc + 1) * 8],
                                    ah_sb[:, hc * 128:(hc + 1) * 128],
                                    bsel_s[:], start=True, stop=True)
                            ahT = work.tile([128, HCN * 8], BF16,
                                            tag="ahT_sb", name="ahT_sb",
                                            bufs=1)
                            nc.vector.tensor_copy(ahT[:], ahT_ps[:])

                            e_ps = psum.tile([BMY, L], FP32, tag="small",
                                             name="e_ps", bufs=1)
                            for hcp in range(2):
                                dt2 = work.tile([128, 2 * NBL], FP8,
                                                tag="dt2", name="dt2",
                                                bufs=2)
                                for i in range(2):
                                    hc = 2 * hcp + i
                                    dp = work.tile([128, NBL], BF16,
                                                   tag="dp", name="dp",
                                                   bufs=2)
                                    eng = (nc.gpsimd if (hcp, i) == (1, 1)
                                           else nc.vector)
                                    eng.tensor_tensor(
                                        dp[:].rearrange("p (b l) -> p b l",
                                                        b=BMY),
                                        p_attT[hc][:].rearrange(
                                            "p (b l) -> p b l", b=BMY),
                                        bcast_free(
                                            ahT[:, hc * 8:(hc + 1) * 8], L),
                                        op=ALU.add)
                                    nc.scalar.activation(
                                        dt2[:, i * NBL:(i + 1) * NBL],
                                        dp[:], AF.Tanh)
                                for b in range(BMY):
                                    lhs = AP(alpha_s[hcp][:].tensor,
                                             alpha_s[hcp][:].offset + b * 32,
                                             [list(alpha_s[hcp][:].ap[0]),
                                              [16, 2], [1, BMY]])
                                    rhs = AP(dt2[:].tensor,
                                             dt2[:].offset + b * L,
                                             [list(dt2[:].ap[0]),
                                              [NBL, 2], [1, L]])
                                    nc.tensor.matmul(
                                        e_ps[:], lhs, rhs,
                                        start=(hcp == 0 and b == 0),
                                        stop=(hcp == 1 and b == BMY - 1),
                                        perf_mode=PM.DoubleRow)

                            # e_ps holds 32*e; |e| <~ 3 so exp(e) is safe
                            # without max-subtraction (alpha ~N(0,.02))
                            u = work.tile([BMY, L], FP32, tag="u", name="u",
                                          bufs=1)
                            ssum = work.tile([BMY, 1], FP32, tag="ssum",
                                             name="ssum", bufs=1)
                            nc.scalar.activation(u[:], e_ps[:], AF.Exp,
                                                 scale=1.0 / W_SCALE,
                                                 accum_out=ssum[:])
                            rinv = work.tile([BMY, 1], FP32, tag="rinv",
                                             name="rinv", bufs=1)
                            nc.vector.reciprocal(rinv[:], ssum[:])
                            nc.vector.tensor_scalar(w_f8[:, 0:L], u[:],
                                                    rinv[:], W_SCALE,
                                                    op0=ALU.mult,
                                                    op1=ALU.mult)

                            wdr = dpool.tile([BMY, LP], FP8, tag="wdr",
                                             name="wdr")
                            nc.sync.dma_start(out=wdr[:], in_=w_f8[:])
                            for b in range(BMY):
                                deng = nc.sync if b % 2 == 0 else nc.scalar
                                deng.dma_start(
                                    out=stat_all[b * 16:(b + 1) * 16,
                                                 b:LP:16],
                                    in_=wdr[b:b + 1, :].rearrange(
                                        "o (g lp) -> (o lp) g", g=NG))

                            # att_res: fp8 DoubleRow, 7 k-tile pairs
                            ar_sb = work.tile([BMY, F], BF16, tag="ar_sb",
                                              name="ar_sb", bufs=1)
                            for fq in range(4):
                                f0 = fq * 512
                                ar_ps = psum.tile([BMY, 512], FP32,
                                                  tag="ar", name="ar_ps",
                                                  bufs=1)
                                for q in range(NG // 2):
                                    lhs = AP(stat_all[:].tensor,
                                             stat_all[:].offset + q * 32,
                                             [list(stat_all[:].ap[0]),
                                              [16, 2], [1, BMY]])
                                    rhs = AP(att_f8_s[:].tensor,
                                             att_f8_s[:].offset
                                             + 2 * q * F + f0,
                                             [list(att_f8_s[:].ap[0]),
                                              [F, 2], [1, 512]])
                                    nc.tensor.matmul(
                                        ar_ps[:], lhs, rhs,
                                        start=(q == 0),
                                        stop=(q == NG // 2 - 1),
                                        perf_mode=PM.DoubleRow)
                                if fq % 2:
                                    nc.vector.tensor_copy(
                                        ar_sb[:, f0:f0 + 512], ar_ps[:])
                                else:
                                    nc.scalar.copy(
                                        ar_sb[:, f0:f0 + 512], ar_ps[:])
                            agA_in = dpool.tile([BMY, F], BF16, tag="agA_in",
                                                name="agA_in")
                            nc.sync.dma_start(out=agA_in[:], in_=ar_sb[:])
                            if no_cc:
                                nc.sync.dma_start(out=agA_out[t][0:BMY, :],
                                                  in_=agA_in[:])
                            else:
                                nc.gpsimd.collective_compute(
                                    "AllGather", ALU.bypass,
                                    replica_groups=RG,
                                    ins=[agA_in.opt()], outs=[agA_out[t][:]])

                            # pre-AG gate work runs in the AllGather window
                            sig3 = work.tile([B, 384], FP32, tag="sig3",
                                             name="sig3", bufs=1)
                            nc.scalar.activation(sig3[:], sums_ps[:, 0:384],
                                                 AF.Tanh, scale=0.5)
                            sitr = work.tile([B, 256], FP32, tag="sitr",
                                             name="sitr", bufs=1)
                            nc.scalar.copy(sitr[:], sums_ps[:, 384:640])
                            a_t = work.tile([B, GC], FP32, tag="a_t",
                                            name="a_t", bufs=1)
                            nc.vector.scalar_tensor_tensor(
                                a_t[:], sig3[:, 128:256], 1.0, c_st[:],
                                op0=ALU.add, op1=ALU.mult)

                            # paired logit for block (t-2)//2 runs in the
                            # AllGather window (last chunk lands in the agH
                            # window below)
                            if t >= 2 and t % 2 == 0:
                                emit_logit_pair((t - 2) // 2, (0, 512))

                            arg_sb = work.tile([B, F], BF16, tag="arg_sb",
                                               name="arg_sb", bufs=1)
                            nc.sync.dma_start(out=arg_sb[:],
                                              in_=agA_out[t][:])
                            # pair two transposes per PSUM tile; one copy
                            # per pair, alternating DVE/ACT
                            arT = work.tile([128, FCN * 64], FP8, tag="arT",
                                            name="arT", bufs=1)
                            for fc2 in range(8):
                                art_ps = psum.tile(
                                    [128, 128], BF16,
                                    tag=("small" if fc2 % 2 else "ctx"),
                                    name="art_ps", bufs=1)
                                # one accumulation group per PSUM tile: the
                                # 2KB zero-region spans both halves, so a
                                # second start=True would re-zero the first
                                for i in range(2):
                                    fc = 2 * fc2 + i
                                    nc.tensor.matmul(
                                        art_ps[:, i * 64:(i + 1) * 64],
                                        arg_sb[:, fc * 128:(fc + 1) * 128],
                                        ident_s[0:B, 0:B],
                                        is_transpose=True,
                                        start=(i == 0), stop=(i == 1),
                                        skip_group_check=True)
                                if fc2 % 2:
                                    nc.vector.tensor_copy(
                                        arT[:, fc2 * 128:(fc2 + 1) * 128],
                                        art_ps[:])
                                else:
                                    nc.scalar.copy(
                                        arT[:, fc2 * 128:(fc2 + 1) * 128],
                                        art_ps[:])

                            # ctx_ps holds 512*ctx (32 from w, 16 from a2c)
                            ctx_ps = psum.tile([B, 256], FP32, tag="ctx",
                                               name="ctx_ps", bufs=1)
                            for fc2 in range(8):
                                lhs = AP(arT[:].tensor,
                                         arT[:].offset + fc2 * 128,
                                         [list(arT[:].ap[0]),
                                          [64, 2], [1, 64]])
                                rhs = AP(a2cT_s[fc2][:].tensor,
                                         a2cT_s[fc2][:].offset,
                                         [list(a2cT_s[fc2][:].ap[0]),
                                          [256, 2], [1, 256]])
                                nc.tensor.matmul(
                                    ctx_ps[:], lhs, rhs, start=(fc2 == 0),
                                    stop=False, perf_mode=PM.DoubleRow)
                            nc.tensor.matmul(ctx_ps[:], ones64[:],
                                             a2c_b_s[:], start=False,
                                             stop=True)

                            itr1 = work.tile([B, GC], FP32, tag="itr1",
                                             name="itr1", bufs=1)
                            nc.vector.scalar_tensor_tensor(
                                itr1[:], ctx_ps[:, 0:128], 1.0 / 512.0,
                                sitr[:, 0:128], op0=ALU.mult, op1=ALU.add)
                            itr2 = work.tile([B, GC], FP32, tag="itr2",
                                             name="itr2", bufs=1)
                            nc.vector.scalar_tensor_tensor(
                                itr2[:], ctx_ps[:, 128:256], 1.0 / 512.0,
                                sitr[:, 128:256], op0=ALU.mult, op1=ALU.add)
                            g_t = work.tile([B, GC], FP32, tag="g_t",
                                            name="g_t", bufs=1)
                            nc.vector.tensor_tensor(g_t[:], itr1[:],
                                                    itr2[:], op=ALU.max)
                            b_t = work.tile([B, GC], FP32, tag="b_t",
                                            name="b_t", bufs=1)
                            nc.vector.scalar_tensor_tensor(
                                b_t[:], sig3[:, 0:128], 1.0, g_t[:],
                                op0=ALU.add, op1=ALU.mult)
                            nc2_t = work.tile([B, GC], FP32, tag="nc2",
                                              name="nc2", bufs=1)
                            nc.vector.tensor_tensor(nc2_t[:], a_t[:],
                                                    b_t[:], op=ALU.add)
                            nc.vector.tensor_scalar(c_st[:], nc2_t[:], 0.5,
                                                    None, op0=ALU.mult)
                            tnc = work.tile([B, GC], FP32, tag="tnc",
                                            name="tnc", bufs=1)
                            nc.scalar.activation(tnc[:], nc2_t[:], AF.Tanh,
                                                 scale=0.5)
                            nh2 = work.tile([B, GC], BF16, tag="nh2",
                                            name="nh2", bufs=1)
                            nc.vector.scalar_tensor_tensor(
                                nh2[:], sig3[:, 256:384], 1.0, tnc[:],
                                op0=ALU.add, op1=ALU.mult)

                            nhT_ps = psum.tile([GC, B], BF16, tag="small",
                                               name="nhT_ps", bufs=1)
                            nc.tensor.transpose(nhT_ps[:], nh2[:],
                                                ident_s[0:B, 0:B])
                            nhT_sb = work.tile([GC, B], BF16, tag="nhT_sb",
                                               name="nhT_sb", bufs=1)
                            nc.vector.tensor_copy(nhT_sb[:], nhT_ps[:])
                            agH_in = dpool.tile([GC, B], BF16, tag="agH_in",
                                                name="agH_in")
                            nc.sync.dma_start(out=agH_in[:], in_=nhT_sb[:])
                            if no_cc:
                                nc.sync.dma_start(out=agH_out[t][0:GC, :],
                                                  in_=agH_in[:])
                            else:
                                nc.gpsimd.collective_compute(
                                    "AllGather", ALU.bypass,
                                    replica_groups=RG,
                                    ins=[agH_in.opt()], outs=[agH_out[t][:]])
                            if t >= 2 and t % 2 == 0:
                                emit_logit_pair((t - 2) // 2, (1024,))
                            # h_{t+1} -> block t//2, half t%2
                            if t % 2 == 0:
                                h2b[t // 2] = hpool.tile(
                                    [128, RCN * 128], BF16, tag="h2",
                                    name=f"h2_{t // 2}")
                                h2f8b[t // 2] = hpool.tile(
                                    [128, RCN * 128], FP8, tag="h2f8",
                                    name=f"h2f8_{t // 2}")
                            blk = h2b[t // 2]
                            dst = AP(blk[:].tensor,
                                     blk[:].offset + (t % 2) * 64,
                                     [list(blk[:].ap[0]), [128, RCN],
                                      [1, 64]])
                            nc.sync.dma_start(
                                out=dst,
                                in_=agH_out[t][:].rearrange(
                                    "(rc rl) b -> rl rc b", rc=RCN))
                            fblk = h2f8b[t // 2]
                            fsrc = AP(blk[:].tensor,
                                      blk[:].offset + (t % 2) * 64,
                                      [list(blk[:].ap[0]), [128, RCN],
                                       [1, 64]])
                            fdst = AP(fblk[:].tensor,
                                      fblk[:].offset + (t % 2) * 64,
                                      [list(fblk[:].ap[0]), [128, RCN],
                                       [1, 64]])
                            nc.vector.tensor_copy(fdst, fsrc)

                            if t == 0:
                                probe("ah0", ah_sb[:], [B, H], BF16)
                                probe("ahT0", ahT[:], [128, HCN * 8], BF16)
                                probe("u0", u[:], [BMY, L], FP32)
                                probe("statall0", stat_all[:],
                                      [128, LP], FP8)
                                probe("ar0", ar_sb[:], [BMY, F], BF16)
                                probe("arT0", arT[:], [128, FCN * 64], BF16)
                                probe("nh20", nh2[:], [B, GC], BF16)
                                probe("agH0", agH_out[0][:], [R, B], BF16)

                        # remaining pairs + stats
                        emit_logit_pair(NPAIR - 1)
                        emit_pair_stats(NPAIR - 1, work)
                        probe("lgb0", lgb[0][:], [128, VP], BF16)

                    # ---------- phase 2 ----------
                    with tc.tile_pool(name=f"p2_{rep}", bufs=2) as p2:
                        agS_in = dpool.tile([128, 2 * NT], FP32,
                                            tag="agS_in", name="agS_in")
                        nc.sync.dma_start(out=agS_in[:, 0:NT],
                                          in_=negm_all[:])
                        nc.sync.dma_start(out=agS_in[:, NT:2 * NT],
                                          in_=s_all[:])
                        if no_cc:
                            nc.sync.dma_start(out=agS_out[0:128, :],
                                              in_=agS_in[:])
                        else:
                            nc.gpsimd.collective_compute(
                                "AllGather", ALU.bypass, replica_groups=RG,
                                ins=[agS_in.opt()], outs=[agS_out[:]])
                        statg = p2.tile([128, NC * 2 * NT], FP32,
                                        tag="statg", name="statg", bufs=1)
                        nc.sync.dma_start(
                            out=statg[:].rearrange("p (r s) -> p r s", r=NC),
                            in_=agS_out[:].rearrange("(r p) s -> p r s",
                                                     r=NC))
                        sview = statg[:].rearrange("p (r s) -> p s r", r=NC)
                        negM = p2.tile([128, NT], FP32, tag="negM",
                                       name="negM", bufs=1)
                        nc.vector.tensor_reduce(negM[:], sview[:, 0:NT, :],
                                                axis=AX.X, op=ALU.min)
                        earg = p2.tile([128, NT * NC], FP32, tag="earg",
                                       name="earg", bufs=1)
                        nc.vector.tensor_tensor(
                            earg[:].rearrange("p (s r) -> p s r", r=NC),
                            bcast_free(negM[:], NC), sview[:, 0:NT, :],
                            op=ALU.subtract)
                        em = p2.tile([128, NT * NC], FP32, tag="em",
                                     name="em", bufs=1)
                        nc.scalar.activation(em[:], earg[:], AF.Exp)
                        sexp = p2.tile([128, NT * NC], FP32, tag="sexp",
                                       name="sexp", bufs=1)
                        nc.vector.tensor_tensor(
                            sexp[:].rearrange("p (s r) -> p s r", r=NC),
                            em[:].rearrange("p (s r) -> p s r", r=NC),
                            sview[:, NT:2 * NT, :], op=ALU.mult)
                        S_t = p2.tile([128, NT], FP32, tag="S_t",
                                      name="S_t", bufs=1)
                        nc.vector.tensor_reduce(
                            S_t[:],
                            sexp[:].rearrange("p (s r) -> p s r", r=NC),
                            axis=AX.X, op=ALU.add)
                        lnS = p2.tile([128, NT], FP32, tag="lnS",
                                      name="lnS", bufs=1)
                        nc.scalar.activation(lnS[:], S_t[:], AF.Ln)
                        logZ = p2.tile([128, NT], FP32, tag="logZ",
                                       name="logZ", bufs=1)
                        nc.vector.scalar_tensor_tensor(
                            logZ[:], negM[:], -1.0, lnS[:], op0=ALU.mult,
                            op1=ALU.add)
                        probe("logZ", logZ[:], [128, NT], FP32)
                        for tt in range(NT):
                            lp_t = p2.tile([128, VP], FP32, tag="p2lp",
                                           name="p2lp", bufs=4)
                            eng = nc.vector if tt % 2 == 0 else nc.gpsimd
                            eng.tensor_scalar(
                                lp_t[:], lgb[tt][:], logZ[:, tt:tt + 1],
                                None, op0=ALU.subtract)
                            deng = nc.sync if tt % 2 == 0 else nc.scalar
                            deng.dma_start(
                                out=out_d[tt * 128:(tt + 1) * 128, :],
                                in_=lp_t[:])

                for rep in range(reps):
                    emit_rep(rep)

    nc.compile()
    return nc, sorted(probes)


_NC_CACHE = {}


def kernel(**inputs):
    """Full-input entry point: returns logp [B, T, V1] float32."""
    from concourse.bass_utils import run_bass_kernel_spmd
    in_maps = host_prep(inputs)
    if "nc" not in _NC_CACHE:
        _NC_CACHE["nc"], _ = build(T, (), reps=1)
    nc = _NC_CACHE["nc"]
    res = run_bass_kernel_spmd(nc, in_maps, list(range(NC)))
    outs = [res.results[c]["logp"] for c in range(NC)]
    full = np.concatenate(outs, axis=1)[:, :V1]          # [T*B, V1]
    logp = full.reshape(T, B, V1).transpose(1, 0, 2)
    return np.ascontiguousarray(logp.astype(np.float32))
